# revision 44
# baseline (speedup 1.0000x reference)
"""Trainium2 Bass kernel: GroupNorm(8) -> 1x1 QKV conv -> 4-head attention
(n=4096, dim_head=32) -> 1x1 out conv, for x[4, 256, 64, 64] f32.

Sharding (8 cores, SPMD, no collectives): core c handles batch c//2 and query
half c%2. Each core receives the full 256x4096 (channels x spatial) slab of
its batch -- spatially ROLLED by 2048 for odd cores, so the program always
computes attention outputs for "queries 0:2048". GroupNorm statistics and
attention are invariant under a permutation of the key/spatial axis, so the
rolled copy yields exactly the outputs of the core's query half.

Device-side layout highlights:
  - sim is computed transposed, [keys, queries], per head at PE row-strip 32h
    (4 heads via tile_position row tiling). exp slices [128 keys, 512 q] are
    split between ScalarE (pairs, via psS) and VectorE (singles, via psD,
    one-op Schraudolph exp in bf16 bit space).
  - attn@V runs with the exp weights as the STATIONARY operand (128-query
    column slices) and V^T (with an appended ones-column for the softmax
    denominator) as the 33-wide MOVING operand: out[q, 33] accumulates over
    key blocks into psO -- 33 PE column-cycles per matmul instead of 512.
  - softmax normalization is then a per-partition reciprocal + broadcast
    multiply on DVE (denominator lands on the same partition as its query),
    producing h^T[q, hid] in SBUF; h^T -> h goes through the DMA XBAR
    transpose (dma_start_transpose), costing no compute-engine time.
  - all matmuls are bf16 (fp32 matmul is 4x slower on TRN2); x ships as bf16
    to halve the startup HBM load.
  - K/Q/V_T and the remaining GroupNorm-ed x chunks are produced inside the
    first attention segment, borrowing idle engine time under the exp stream;
    attn@V matmuls lag the exp stream by a couple of chunks so segment
    epilogues can drain psO without stalling the PE FIFO.
"""

import ml_dtypes
import numpy as np

HEADS, DH, G, EPS = 4, 32, 8, 1e-5
B, C, HW = 4, 256, 64
N = HW * HW          # 4096 spatial positions (keys)
NQ = N // 2          # 2048 queries per core
NKB = N // 128       # 32 key blocks
NQB = NQ // 512      # 4 query blocks of 512
CT = C // 128        # 2 channel tiles
HID = HEADS * DH     # 128

_BUILT = {}

# tuning knobs (read at build time)
CFG = {
    "flush_ci": 10,     # tile index where previous segment epilogue flushes
    "np": 18,           # ScalarE exp pairs per steady pair-segment
    "np0": 24,          # ScalarE exp pairs in segment 0
    "psd_bufs": 3,      # psD pool buffers (psd_bufs + pso_bufs == 4)
    "pso_bufs": 1,      # psO pool buffers
    "prefetch": False,  # pre-attention K1/V0/V1 via ScalarE copies
    "tail_split": False,  # final segment: half-granular yt + output DMA
    "tail_lag": 26,     # tiles between qb transposes and out-projection MMs
    "yt_lag": 30,       # tiles until the bias-add + output DMA
    "yp_psS": False,    # out-projection psum via the ScalarE-side pool
    "bn_pool": False,   # GroupNorm bn_stats on GpSimd (SBUF-only op)
    "oacc_hi": 20,      # attn@V emitter queue high-water mark
    "fused_epilogue": True,  # per-qsub epilogue/out-projection pipelining
    "xchunks": 8,       # DMA chunk count for the x load
    "ocp": None,        # None | "dve" | "act": copy psO->SBUF first
    "xn_pool": False,   # GroupNorm-apply on GpSimd instead of VectorE
    "ebufs": 13,        # exp-output tile double-buffering depth
    "epi_act": False,   # epilogue normalize-scale on ScalarE instead of DVE
}


_MAX_INST_WAITS = 1


def _patch_tail_drain(tile_mod):
    """Walrus codegen on this toolchain only supports a small number of sync
    waits per ISA instruction. Two patches:
    1. every committed instruction with too many waits gets the excess hoisted
       onto same-engine nops emitted immediately before it (same stream
       position => identical semantics);
    2. the TileContext tail drain (one wait per engine + DMA lane, >8 total)
       is split one wait per SP nop."""
    if getattr(tile_mod.TileContext, "_drain_patched", False):
        return
    import bass_rust
    from concourse.vector_clock import ScopedClock

    _orig_add_instruction = tile_mod.TileContext._add_instruction

    _SELF_WAIT_OK = ("InstActivation",)

    def _add_with_wait_split(self, inst):
        si = getattr(inst, "sync_info", None)
        if (
            si is not None
            and si.on_wait
            and type(inst).__name__ in _SELF_WAIT_OK
        ):
            # engine queues are strict FIFO: a data op's wait on its own
            # engine's sem is redundant (prior same-engine ops complete in
            # order before it) -- dropping it avoids a split nop per op
            eng_name = str(inst.engine).split(".")[-1]
            kept = [
                w for w in si.on_wait
                if w.ant_name.rsplit("_", 1)[0] != eng_name
            ]
            if len(kept) != len(si.on_wait):
                import bass_rust as _br
                inst.sync_info = _br.SyncInfo(
                    on_wait=kept, on_update=list(si.on_update)
                )
                si = inst.sync_info
        if si is not None and len(si.on_wait) > _MAX_INST_WAITS:
            waits = list(si.on_wait)
            keep, excess = waits[: _MAX_INST_WAITS], waits[_MAX_INST_WAITS :]
            eng = self.nc.engines[inst.engine]
            for w in excess:  # NoOps only support a single wait slot
                nop = eng.nop(nofuse=True, hint="wait_split")
                nop.ins.sync_info = bass_rust.SyncInfo(on_wait=[w], on_update=[])
            inst.sync_info = bass_rust.SyncInfo(
                on_wait=keep, on_update=list(si.on_update)
            )
        return _orig_add_instruction(self, inst)

    tile_mod.TileContext._add_instruction = _add_with_wait_split

    def _patched(self, tick_clock, wait_clock):
        nop = self.nc.sync.nop(nofuse=True, hint="pre_drain_wait_split")
        wait_clock.add_sem_waits(nop.ins, ScopedClock({None: tick_clock.global_clock}))
        si = nop.ins.sync_info
        waits = list(si.on_wait) if si is not None else []
        if len(waits) > 1:
            nop.ins.sync_info = bass_rust.SyncInfo(
                on_wait=waits[:1], on_update=list(si.on_update)
            )
            for i in range(1, len(waits)):
                n2 = self.nc.sync.nop(nofuse=True, hint="pre_drain_wait_split")
                n2.ins.sync_info = bass_rust.SyncInfo(on_wait=[waits[i]], on_update=[])
        self.nc.sync.drain()
        self.nc.all_engine_barrier()
        assert self.sems is not None
        popped = self.nc._tile_sem_poison_stack.pop()
        assert popped is self._sem_poison
        self.nc.clear_and_free_semaphores(list(self.sems.allocated().values()))
        # no trailing all_engine_barrier: NEFF completion already requires the
        # SP stream (which performs the sem clears) to finish, and every other
        # engine is quiesced by the barrier above

    tile_mod.TileContext._drain_and_barrier = _patched
    tile_mod.TileContext._drain_patched = True


def build_nc():
    import concourse.bass as bass
    import concourse.mybir as mybir
    import concourse.tile as tile

    _patch_tail_drain(tile)
    f32 = mybir.dt.float32
    bf16 = mybir.dt.bfloat16
    AF = mybir.ActivationFunctionType
    ALU = mybir.AluOpType

    nc = bass.Bass()
    x_d = nc.declare_dram_parameter("x", [CT, 128, N], bf16, isOutput=False)
    wq_d = nc.declare_dram_parameter("wq", [CT, 128, 3 * HID], f32, isOutput=False)
    wo_d = nc.declare_dram_parameter("wo", [HID, C], f32, isOutput=False)
    gb_d = nc.declare_dram_parameter("gb", [128, CT, 3], f32, isOutput=False)
    gmask_d = nc.declare_dram_parameter("gmask", [128, CT, G], f32, isOutput=False)
    sel_d = nc.declare_dram_parameter("selT", [G, CT, 128], f32, isOutput=False)
    out_d = nc.declare_dram_parameter("out", [CT, 128, NQ], bf16, isOutput=True)

    with tile.TileContext(nc) as tc:
        with (
            tc.tile_pool(name="big", bufs=1) as big,
            tc.tile_pool(name="work", bufs=2) as work,
            tc.tile_pool(name="epool", bufs=CFG["ebufs"]) as epool,
            tc.tile_pool(name="e2pool", bufs=CFG["ebufs"]) as e2pool,
            tc.tile_pool(name="hpool", bufs=2) as hpool,
            tc.tile_pool(name="psS", bufs=2, space="PSUM") as psS,  # 2 x 2 banks
            tc.tile_pool(name="psD", bufs=CFG["psd_bufs"], space="PSUM") as psD,
            tc.tile_pool(name="psO", bufs=CFG["pso_bufs"], space="PSUM") as psO,
        ):
            # ---------------- load inputs ----------------
            x_sb = big.tile([128, CT, N], bf16)
            nxc = CFG["xchunks"] // CT
            xw = N // nxc
            for t in range(CT):
                for j in range(nxc):
                    nc.sync.dma_start(
                        x_sb[:, t, j * xw : (j + 1) * xw],
                        x_d[t, :, j * xw : (j + 1) * xw],
                    )
            wq_sb = big.tile([128, CT, 3 * HID], f32)
            nc.sync.dma_start(wq_sb[:, 0], wq_d[0])
            nc.sync.dma_start(wq_sb[:, 1], wq_d[1])
            wo_sb = big.tile([HID, C], f32)
            nc.sync.dma_start(wo_sb[:], wo_d[:])
            gb_sb = big.tile([128, CT, 3], f32)
            nc.sync.dma_start(gb_sb[:], gb_d[:])
            gmask_sb = big.tile([128, CT, G], f32)
            nc.sync.dma_start(gmask_sb[:], gmask_d[:])
            sel_sb = big.tile([G, CT, 128], f32)
            nc.sync.dma_start(sel_sb[:], sel_d[:])

            wq_bf = big.tile([128, CT, 3 * HID], bf16)
            nc.scalar.activation(wq_bf[:], wq_sb[:], AF.Identity)
            wo_bf = big.tile([HID, C], bf16)
            nc.scalar.activation(wo_bf[:], wo_sb[:], AF.Identity)

            # ---------------- group norm ----------------
            # per-channel mean/var via DVE bn_stats (512-col chunks, overlaps
            # the x DMAs); T = [mean_c, E[x^2]_c]
            # Per-channel sums: every third 512-col chunk goes to the (idle)
            # ScalarE as Square/Identity passes with accum_out; the rest stay
            # on VectorE bn_stats. T[:,t] = [sum x, sum x^2] per channel
            # (later normalized by 1/(32*4096) inside the group mask weights).
            T_sb = work.tile([128, CT, 2], f32, tag="gnT")
            asc = work.tile([128, 512], bf16, tag="asc")  # ACT scratch
            for t in range(CT):
                bst = work.tile([128, 8, nc.vector.BN_STATS_DIM], f32, tag="gnbst")
                acc = work.tile([128, 4, 2], f32, tag="gnacc")
                na = 0
                for j in range(8):
                    sl = x_sb[:, t, j * 512 : (j + 1) * 512]
                    if j % 3 == 2:
                        nc.scalar.activation(
                            asc[:], sl, AF.Identity, accum_out=acc[:, na, 0:1]
                        )
                        nc.scalar.activation(
                            asc[:], sl, AF.Square, accum_out=acc[:, na, 1:2]
                        )
                        na += 1
                    else:
                        nc.vector.bn_stats(out=bst[:, j, :], in_=sl)
                # aggregate the VectorE chunks (6 of 8): bn_aggr needs a
                # contiguous bst block, so compact first
                nc.vector.tensor_copy(bst[:, 2, :], bst[:, 3, :])
                nc.vector.tensor_copy(bst[:, 3, :], bst[:, 4, :])
                nc.vector.tensor_copy(bst[:, 4, :], bst[:, 6, :])
                nc.vector.tensor_copy(bst[:, 5, :], bst[:, 7, :])
                mv = work.tile([128, nc.vector.BN_AGGR_DIM], f32, tag="gnmv")
                nc.vector.bn_aggr(out=mv[:], in_=bst[:, 0:6, :])
                # bn chunk sums: n1*mean, n1*(var + mean^2) with n1 = 6*512
                n1 = 6 * 512.0
                nc.vector.tensor_mul(T_sb[:, t, 1:2], mv[:, 0:1], mv[:, 0:1])
                nc.vector.tensor_tensor(
                    T_sb[:, t, 1:2], T_sb[:, t, 1:2], mv[:, 1:2], ALU.add
                )
                nc.vector.tensor_scalar(
                    T_sb[:, t, 1:2], T_sb[:, t, 1:2],
                    scalar1=n1, scalar2=None, op0=ALU.mult,
                )
                nc.vector.tensor_scalar(
                    T_sb[:, t, 0:1], mv[:, 0:1],
                    scalar1=n1, scalar2=None, op0=ALU.mult,
                )
                # add the ScalarE accumulator chunks
                for c in range(2):
                    nc.vector.tensor_tensor(
                        T_sb[:, t, c : c + 1], T_sb[:, t, c : c + 1],
                        acc[:, 0, c : c + 1], ALU.add,
                    )
                    nc.vector.tensor_tensor(
                        T_sb[:, t, c : c + 1], T_sb[:, t, c : c + 1],
                        acc[:, 1, c : c + 1], ALU.add,
                    )
            # group stats: [G, 2] = sum over channels-in-group / (32*4096)
            # (prologue borrows the psO banks before attention needs them)
            ps_pro = psO.tile([128, 512], f32, tag="oacc", name="pro")
            stats_ps = ps_pro[0:G, 0:2]
            for t in range(CT):
                nc.tensor.matmul(
                    stats_ps, gmask_sb[:, t, :], T_sb[:, t, :],
                    start=(t == 0), stop=(t == CT - 1),
                )
            # var = E[x^2] - mean^2 ; rstd = 1/sqrt(var+eps)
            stats_sb = work.tile([G, 2], f32, tag="gnstats")  # [mean, rstd]
            stats_tmp = work.tile([G, 2], f32, tag="gnstats_raw")
            msq = work.tile([G, 1], f32, tag="gnmsq")
            nc.vector.tensor_copy(stats_tmp[:], stats_ps)
            nc.vector.tensor_copy(stats_sb[:, 0:1], stats_tmp[:, 0:1])
            nc.vector.tensor_mul(msq[:], stats_tmp[:, 0:1], stats_tmp[:, 0:1])
            nc.vector.tensor_tensor(
                stats_sb[:, 1:2], stats_tmp[:, 1:2], msq[:], ALU.subtract
            )
            eps_sb = work.tile([G, 1], f32, tag="gneps")
            nc.vector.memset(eps_sb[:], EPS)
            nc.scalar.activation(
                stats_sb[:, 1:2], stats_sb[:, 1:2], AF.Sqrt, bias=eps_sb[:]
            )
            nc.vector.reciprocal(stats_sb[:, 1:2], stats_sb[:, 1:2])
            # broadcast to channels; gamma/beta fold into the projections
            scs = []
            bc_ps = []
            for t in range(CT):
                bc = ps_pro[:, 64 + 2 * t : 66 + 2 * t]  # [mean_c, rstd_c]
                nc.tensor.matmul(bc, sel_sb[:, t, :], stats_sb[:], start=True, stop=True)
                bc_ps.append(bc)
            for t in range(CT):
                sc = work.tile([128, 2], f32, tag=f"gnsc{t}")  # [scale_c, bias_c]
                nc.vector.tensor_mul(sc[:, 0:1], gb_sb[:, t, 0:1], bc_ps[t][:, 1:2])
                nc.vector.tensor_mul(sc[:, 1:2], bc_ps[t][:, 0:1], sc[:, 0:1])
                nc.vector.tensor_tensor(
                    sc[:, 1:2], gb_sb[:, t, 1:2], sc[:, 1:2], ALU.subtract
                )
                scs.append(sc)

            # ---------------- QKV projections ----------------
            # GroupNorm is folded into the projections: the per-channel scale
            # multiplies the projection weights (wq_sc), the per-channel bias
            # becomes a per-hid constant added during the psum->SBUF copies
            # (K/Q) or via a 1-row accumulating matmul (V^T). xn itself is
            # never materialized.
            # Only K block 0 and Q block 0 are produced up front; the rest is
            # interleaved into the first attention segment so ScalarE starts
            # exp-ing as early as possible.
            k_bf = big.tile([128, N], bf16)      # [hid, keys] head-major
            q_bf = big.tile([128, NQ], bf16)     # [hid, queries] (pre-scaled W)
            v_bf = big.tile([128, NKB, HID], bf16)  # [key128, kb, hid] via XBAR transpose
            vh_bf = big.tile([128, N], bf16)        # [hid, keys] V projection
            wq_sc = big.tile([128, CT, 3 * HID], bf16)  # weights x GN scale
            kq_bias = big.tile([128, 3], f32)   # [beta_K | beta_Q | beta_V]
            # K part first: the critical path to the first sim runs through
            # wq_sc[K] -> K0 -> sims; Q/V parts follow
            for p0 in (HID, 0, 2 * HID):
                for t in range(CT):
                    with nc.allow_low_precision(
                        reason="single rounding: f32 weights x GN scale -> bf16"
                    ):
                        nc.vector.tensor_scalar(
                            wq_sc[:, t, p0 : p0 + HID],
                            wq_sb[:, t, p0 : p0 + HID],
                            scalar1=scs[t][:, 0:1], scalar2=None, op0=ALU.mult,
                        )
            # beta_P[hid] = sum_c W_P[c, hid] * gnbias_c via PE; K/Q as [hid,1]
            # columns (per-partition add in the psum->SBUF copy), V as a
            # [1,hid] row (rank-1 accumulating matmul into V^T)
            bvec = work.tile([128, CT], bf16, tag="bvec")
            for t in range(CT):
                nc.vector.tensor_copy(bvec[:, t : t + 1], scs[t][:, 1:2])
            pb_ps = psD.tile([128, 512], f32, tag="simd", name="pbias")
            # col 0: beta_K, col 1: beta_Q, col 2: beta_V (per-hid columns)
            for p, w0 in enumerate([HID, 0, 2 * HID]):
                for t in range(CT):
                    nc.tensor.matmul(
                        pb_ps[:, p : p + 1],
                        wq_bf[:, t, w0 : w0 + HID],
                        bvec[:, t : t + 1],
                        start=(t == 0), stop=(t == CT - 1),
                    )
            nc.vector.tensor_copy(kq_bias[:], pb_ps[:, 0:3])
            ones1 = big.tile([128, 1], bf16)
            nc.vector.memset(ones1[:], 1.0)

            def emit_k(cb):
                kp = psD.tile([128, 512], f32, tag="simd", name=f"kp{cb}")[:, :]
                for t in range(CT):
                    nc.tensor.matmul(
                        kp, wq_sc[:, t, HID : 2 * HID],
                        x_sb[:, t, cb * 512 : (cb + 1) * 512],
                        start=(t == 0), stop=(t == CT - 1),
                    )
                if cb == 0:  # ScalarE is idle pre-attention; keep DVE free.
                    # kb=0's columns land first so tile 0's sims start early
                    nc.scalar.activation(
                        k_bf[:, 0:128], kp[:, 0:128], AF.Identity,
                        bias=kq_bias[:, 0:1],
                    )
                    nc.scalar.activation(
                        k_bf[:, 128:512], kp[:, 128:512], AF.Identity,
                        bias=kq_bias[:, 0:1],
                    )
                else:
                    nc.vector.tensor_scalar(
                        k_bf[:, cb * 512 : (cb + 1) * 512], kp,
                        scalar1=kq_bias[:, 0:1], scalar2=None, op0=ALU.add,
                    )

            def emit_q(cb):
                qp = psD.tile([128, 512], f32, tag="simd", name=f"qp{cb}")[:, :]
                for t in range(CT):
                    nc.tensor.matmul(
                        qp, wq_sc[:, t, 0:HID],
                        x_sb[:, t, cb * 512 : (cb + 1) * 512],
                        start=(t == 0), stop=(t == CT - 1),
                    )
                if cb == 0:
                    nc.scalar.activation(
                        q_bf[:, 0:512], qp, AF.Identity, bias=kq_bias[:, 1:2]
                    )
                else:
                    nc.vector.tensor_scalar(
                        q_bf[:, cb * 512 : (cb + 1) * 512], qp,
                        scalar1=kq_bias[:, 1:2], scalar2=None, op0=ALU.add,
                    )

            def emit_v(cb, use_act=False):
                # V as [hid, keys] (same shape as K), bias folded into the
                # psum->SBUF copy, then per-128 XBAR transposes into v_bf
                vp = psD.tile([128, 512], f32, tag="simd", name=f"vp{cb}")[:, :]
                for t in range(CT):
                    nc.tensor.matmul(
                        vp, wq_sc[:, t, 2 * HID : 3 * HID],
                        x_sb[:, t, cb * 512 : (cb + 1) * 512],
                        start=(t == 0), stop=(t == CT - 1),
                    )
                c_sl = slice(cb * 512, (cb + 1) * 512)
                if use_act:
                    nc.scalar.activation(
                        vh_bf[:, c_sl], vp, AF.Identity, bias=kq_bias[:, 2:3]
                    )
                else:
                    nc.vector.tensor_scalar(
                        vh_bf[:, c_sl], vp,
                        scalar1=kq_bias[:, 2:3], scalar2=None, op0=ALU.add,
                    )
                for kb in range(4 * cb, 4 * cb + 4):
                    nc.sync.dma_start_transpose(
                        v_bf[:, kb, :], vh_bf[:, kb * 128 : (kb + 1) * 128]
                    )
            zero_sb = big.tile([1, 512], bf16)
            nc.vector.memset(zero_sb[:], 0.0)
            emit_k(0)
            emit_q(0)


            # ---------------- attention + out-projection ----------------
            # Segment = (query block of 512, head pair): 64 slices (kb = i//2,
            # head = 2*pair + i%2), each slice one [128 keys x 512 q] sim
            # plane. Sim tiles hold 3 (or 2) planes in the 6-bank psA pool and
            # are consumed whole either by ScalarE (exact exp) or VectorE
            # (one-op Schraudolph exp in bf16 bit space, int16 out) -- bigger
            # tiles amortize each engine's fixed access cost. attn@V: per
            # slice 4 matmuls (one per 128-query block), exp weights
            # stationary, V^T_ext (32 V + 1 ones col) moving:
            # psO[q, 66*qs+33*hi] += e[:, qs]^T @ v_ext accumulated over kb;
            # one pair's worth fits a single PSUM bank.
            SLICES = NKB * 2  # 64 per segment

            def seg_pattern(npair):
                # npair 2-slice ScalarE exp tiles (psS) interleaved with
                # 1-slice VectorE Schraudolph tiles (psD); each stream is
                # independently double-buffered. Head tiles feed ScalarE (the
                # epilogue owns VectorE's stream head); tail tiles feed
                # VectorE so the next epilogue is not gated on ScalarE's lag.
                nd = SLICES - 2 * npair
                pat, a, d = ["A", "A"], 2, 0
                while a < npair or d < nd - 2:
                    if (d * npair >= a * nd and a < npair) or d >= nd - 2:
                        pat.append("A")
                        a += 1
                    else:
                        pat.append("D")
                        d += 1
                pat += ["D", "D"]
                return pat

            def make_epilogue(qb, pair, oacc, hT):
                # per qsub: 1/denominator (on the query's own partition),
                # broadcast-multiply O^T into this pair's half of hT
                def _emit(zmm):
                    ov = oacc[:, 0 : 4 * 66].rearrange("p (q c) -> p q c", c=66)
                    rcp = work.tile([128, 4, 2], f32, tag="rcp")
                    with nc.allow_low_precision(
                        reason="softmax denom reciprocal feeds bf16 output"
                    ):
                        nc.vector.reciprocal(rcp[:], ov[:, :, 64:66])
                        nc.vector.tensor_tensor(
                            hT[:, :, 64 * pair : 64 * pair + 64].rearrange(
                                "p q (h c) -> p q h c", c=DH
                            ),
                            ov[:, :, 0:64].rearrange(
                                "p q (h c) -> p q h c", c=DH
                            ),
                            rcp[:].unsqueeze(3).broadcast_to([128, 4, 2, DH]),
                            ALU.mult,
                        )
                    if zmm is not None:
                        zmm()
                return _emit

            def make_qb_transp(qb, hT):
                def _emit(zmm):
                    h_sb = hpool.tile([128, 512], bf16, tag="h")
                    for qs in range(4):
                        nc.sync.dma_start_transpose(
                            h_sb[:, qs * 128 : (qs + 1) * 128], hT[:, qs]
                        )
                    return h_sb
                return _emit

            def make_qb_mms(qb, h_box, yp_box):
                # out-projection over the transposed h (emitted a few tiles
                # after the transposes so their DMA latency stays off the PE
                # FIFO's critical path)
                def _emit(zmm):
                    h_sb = h_box[0]
                    if CFG["yp_psS"]:
                        ypt = psS.tile([128, 2, 512], f32, tag="sim", name=f"yp{qb}")
                        yps = [ypt[:, oc, :] for oc in range(CT)]
                    else:
                        yps = [
                            psD.tile([128, 512], f32, tag="simd", name=f"yp{qb}_{oc}")[:, :]
                            for oc in range(CT)
                        ]
                    yp_box.append(yps)
                    for qs in range(4):
                        for oc in range(CT):
                            nc.tensor.matmul(
                                yps[oc][:, qs * 128 : (qs + 1) * 128],
                                wo_bf[:, oc * 128 : (oc + 1) * 128],
                                h_sb[:, qs * 128 : (qs + 1) * 128],
                                start=True, stop=True,
                            )
                return _emit

            def make_qb_tail(qb, yp_box, use_act):
                def _emit(zmm):  # zmm unused: only the epilogue re-zeroes psO
                    yps = yp_box[0]
                    q0 = qb * 512
                    if use_act and CFG["tail_split"]:
                        # tail: ScalarE is idle after the last exp; halves let
                        # the output DMA start while qs2/3 are still in flight
                        yts = [
                            work.tile([128, 512], bf16, tag=f"yt{oc}", name=f"yt{oc}")
                            for oc in range(CT)
                        ]
                        for half in range(2):
                            c_sl = slice(half * 256, half * 256 + 256)
                            for oc in range(CT):
                                nc.scalar.activation(
                                    yts[oc][:, c_sl], yps[oc][:, c_sl],
                                    AF.Identity, bias=gb_sb[:, oc, 2:3],
                                )
                                nc.sync.dma_start(
                                    out_d[oc, :, q0 + half * 256 : q0 + half * 256 + 256],
                                    yts[oc][:, c_sl],
                                )
                    else:
                        for oc in range(CT):
                            yt = work.tile([128, 512], bf16, tag="yt")
                            if use_act:
                                nc.scalar.activation(
                                    yt[:], yps[oc][:], AF.Identity,
                                    bias=gb_sb[:, oc, 2:3],
                                )
                            else:
                                nc.vector.tensor_scalar(
                                    yt[:], yps[oc][:], scalar1=gb_sb[:, oc, 2:3],
                                    scalar2=None, op0=ALU.add,
                                )
                            nc.sync.dma_start(
                                out_d[oc, :, q0 : q0 + 512], yt[:]
                            )
                return _emit

            pending = []    # deferred epilogue / qb-tail emitters
            oaccq = []      # attn@V emitters lagging the exp stream

            def emit_sim(s_idx, tile_ap, plane, qb, pair):
                kb, hi = s_idx // 2, s_idx % 2
                h = 2 * pair + hi
                q_sl = slice(qb * 512, (qb + 1) * 512)
                nc.tensor.matmul(
                    tile_ap[:, plane, :],
                    k_bf[32 * h : 32 * h + 32, kb * 128 : (kb + 1) * 128],
                    q_bf[32 * h : 32 * h + 32, q_sl],
                    start=True, stop=True,
                    tile_position=(32 * h, 0),
                )

            def make_oacc(s_idx, e_ap, oacc, pair):
                kb, hi = s_idx // 2, s_idx % 2
                h = 2 * pair + hi
                def _emit(last=False):
                    # PSUM adds commute, so attn@V order across slices is
                    # free; stop (clearing the bank's group-started state)
                    # must ride the final matmul actually emitted
                    for qs in range(4):
                        col = 66 * qs + 32 * hi
                        nc.tensor.matmul(
                            oacc[:, col : col + 32],
                            e_ap[:, qs * 128 : (qs + 1) * 128],
                            v_bf[:, kb, 32 * h : 32 * h + 32],
                            start=False, stop=False,
                        )
                        nc.tensor.matmul(
                            oacc[:, 66 * qs + 64 + hi : 66 * qs + 65 + hi],
                            e_ap[:, qs * 128 : (qs + 1) * 128],
                            ones1[:],
                            start=False,
                            stop=(last and qs == 3),
                        )
                return _emit

            hT = None
            for sg in range(2 * NQB):
                qb, pair = sg // 2, sg % 2
                seg0 = sg == 0
                oacc = psO.tile([128, 512], f32, tag="oacc", name=f"o{sg}")

                def zmm(oacc=oacc):
                    # one start=True matmul filling the whole bank resets its
                    # pending-zero state in one shot, so the interleaved
                    # per-(head, qsub) groups can then pure-accumulate (a
                    # start per group would wipe the others' first block)
                    nc.tensor.matmul(
                        oacc[:], zero_sb[:, 0:128], zero_sb[:],
                        start=True, stop=False,
                    )

                if seg0:
                    zmm()
                if pair == 0:
                    hT = hpool.tile([128, 4, HID], bf16, tag="hT")
                pat = seg_pattern(CFG["np0"] if seg0 else CFG["np"])
                i = 0
                for ci, kind in enumerate(pat):
                    ts_ = 2 if kind == "A" else 1
                    if kind == "A":
                        tile = psS.tile([128, 2, 512], f32, tag="sim", name="sim")
                    else:
                        tile = psD.tile([128, 512], f32, tag="simd", name="simd").unsqueeze(1)
                    for s in range(ts_):
                        emit_sim(i + s, tile, s, qb, pair)
                    due = [
                        (o, fn) for o, fn in pending
                        if ci >= CFG["flush_ci"] + o
                    ]
                    if due:
                        # previous segment's epilogue: its DVE ops lead the
                        # queue; the freed psO bank is re-zeroed (zmm) right
                        # after its last read retires
                        for o, fn in due:
                            fn(zmm if not seg0 else None)
                        pending = [p for p in pending if p not in due]
                    if seg0:
                        # produce the next V / K column blocks, overlapped
                        # with the exp stream (V leads: its transposes add
                        # DMA latency before the attn@V needs it)
                        for kb in range((i + 1) // 2, (i + ts_ + 1) // 2):
                            if kb % 4 == 0 and 0 <= kb // 4 < 8:
                                emit_v(kb // 4)
                            if kb % 4 == 2 and 0 < kb // 4 + 1 < 8:
                                emit_k(kb // 4 + 1)
                    if qb < NQB - 1 and pair == 1 and ci == 10:
                        emit_q(qb + 1)
                    if kind == "A":
                        e = epool.tile([128, 2, 512], bf16, tag="e")
                        nc.scalar.activation(e[:], tile[:], AF.Exp)
                        for s in range(2):
                            oaccq.append(make_oacc(i + s, e[:, s, :], oacc, pair))
                    else:
                        e2 = e2pool.tile([128, 512], mybir.dt.int16, tag="e2")
                        with nc.allow_low_precision(
                            reason="Schraudolph bf16 exp on DVE; softmax renormalizes"
                        ):
                            nc.vector.tensor_scalar(
                                e2[:], tile[:, 0, :],
                                scalar1=128.0 / 0.6931471805599453,
                                scalar2=(127.0 - 0.043) * 128.0,
                                op0=ALU.mult, op1=ALU.add,
                            )
                        oaccq.append(make_oacc(i, e2.bitcast(bf16), oacc, pair))
                    i += ts_
                    while len(oaccq) > CFG["oacc_hi"]:
                        oaccq.pop(0)()
                while oaccq:
                    fn_, last_ = oaccq.pop(0), not oaccq
                    fn_(last_)
                pending.append((0, make_epilogue(qb, pair, oacc, hT)))
                if pair == 1:
                    h_box, yp_box = [], []
                    tp = make_qb_transp(qb, hT)
                    pending.append(
                        (0, lambda z, tp=tp, h_box=h_box: h_box.append(tp(None)))
                    )
                    pending.append((CFG["tail_lag"], make_qb_mms(qb, h_box, yp_box)))
                    pending.append(
                        (CFG["yt_lag"], make_qb_tail(qb, yp_box, use_act=(qb == NQB - 1)))
                    )
            for _, fn in pending:
                fn(None)
    return nc


def _prep_shared(w_qkv, w_out, b_out, gamma, beta):
    scale = DH ** -0.5
    wqkvT = np.ascontiguousarray(w_qkv.T).astype(np.float32).copy()  # [C, 384]
    wqkvT[:, :HID] *= scale
    wq = np.ascontiguousarray(wqkvT.reshape(CT, 128, 3 * HID))
    wo = np.ascontiguousarray(w_out.T).astype(np.float32)            # [HID, C]
    gb = np.stack(
        [
            np.asarray(gamma, np.float32).reshape(CT, 128).T,
            np.asarray(beta, np.float32).reshape(CT, 128).T,
            np.asarray(b_out, np.float32).reshape(CT, 128).T,
        ],
        axis=-1,
    )  # [128, CT, 3]
    gmask = np.zeros((128, CT, G), np.float32)
    sel = np.zeros((G, CT, 128), np.float32)
    for t in range(CT):
        for p in range(128):
            g = (t * 128 + p) // (C // G)
            gmask[p, t, g] = 1.0 / ((C // G) * N)
            sel[g, t, p] = 1.0
    return wq, wo, gb, gmask, sel


def _run(inputs, trace=False):
    from concourse.bass_utils import run_bass_kernel_spmd

    x = np.asarray(inputs["x"], np.float32)
    wq, wo, gb, gmask, sel = _prep_shared(
        np.asarray(inputs["w_qkv"], np.float32),
        np.asarray(inputs["w_out"], np.float32),
        np.asarray(inputs["b_out"], np.float32),
        np.asarray(inputs["gamma"], np.float32),
        np.asarray(inputs["beta"], np.float32),
    )
    if "nc" not in _BUILT:
        _BUILT["nc"] = build_nc()
    nc = _BUILT["nc"]

    in_maps = []
    for core in range(8):
        b_idx, qh = core // 2, core % 2
        xb = x[b_idx].reshape(C, N)
        if qh:
            xb = np.roll(xb, -NQ, axis=1)
        in_maps.append(
            {
                "x": np.ascontiguousarray(
                    xb.reshape(CT, 128, N).astype(ml_dtypes.bfloat16)
                ),
                "wq": wq, "wo": wo, "gb": gb, "gmask": gmask, "selT": sel,
            }
        )
    res = run_bass_kernel_spmd(
        nc, in_maps, core_ids=list(range(8)), trace=trace
    )
    out = np.empty((B, C, N), np.float32)
    for core in range(8):
        b_idx, qh = core // 2, core % 2
        y = res.results[core]["out"].astype(np.float32).reshape(C, NQ)
        out[b_idx, :, qh * NQ : (qh + 1) * NQ] = y
    return out.reshape(B, C, HW, HW), res


def kernel(**inputs) -> np.ndarray:
    out, _ = _run(inputs, trace=False)
    return out


# revision 45
# speedup vs baseline: 1.0126x; 1.0126x over previous
"""Trainium2 Bass kernel: GroupNorm(8) -> 1x1 QKV conv -> 4-head attention
(n=4096, dim_head=32) -> 1x1 out conv, for x[4, 256, 64, 64] f32.

Sharding (8 cores, SPMD, no collectives): core c handles batch c//2 and query
half c%2. Each core receives the full 256x4096 (channels x spatial) slab of
its batch -- spatially ROLLED by 2048 for odd cores, so the program always
computes attention outputs for "queries 0:2048". GroupNorm statistics and
attention are invariant under a permutation of the key/spatial axis, so the
rolled copy yields exactly the outputs of the core's query half.

Device-side layout highlights:
  - GroupNorm is folded into the QKV projections: per-channel scale multiplies
    the (f32) projection weights once on-device; per-channel bias becomes a
    per-hid constant added during the psum->SBUF copies (K/Q/V). The
    normalized activation tensor is never materialized. Channel statistics
    come from VectorE bn_stats chunks plus ScalarE Square/Identity accum_out
    passes (ScalarE is idle while x streams in).
  - sim is computed transposed, [keys, queries], per head at PE row-strip 32h
    (tile_position row tiling). Work is organized as 8 pair-segments
    (query-block x head-pair) of 64 slices; each slice is a [128 keys x 512 q]
    sim plane. Exp is split between ScalarE (2-slice psS tiles, exact exp) and
    VectorE (1-slice psD tiles, one-op Schraudolph exp in bf16 bit space);
    both streams are independently double/triple buffered.
  - attn@V runs with the exp weights as the STATIONARY operand (128-query
    column slices) and V^T as the 32-wide MOVING operand (plus a 1-column
    ones matmul for the softmax denominator): psO[q, 66*qs+32*hi] += e^T @ v,
    33 PE column-cycles per slice-quarter instead of 512. One pair-segment
    fits a single PSUM bank; a single full-bank zeroing matmul per segment
    resets the pending-zero state so the interleaved per-(head, qsub)
    accumulation groups can pure-accumulate.
  - softmax normalization is one strided reciprocal + one broadcast multiply
    per segment (denominators land on their query's partition); h^T -> h goes
    through the DMA XBAR transpose (dma_start_transpose), costing no
    compute-engine time. V^T is likewise produced by XBAR transposes of a
    hid-major V projection. Out-projection matmuls are deferred ~26 tiles so
    transpose DMA latency never blocks the PE FIFO.
  - all matmuls are bf16 (fp32 matmul is 4x slower on TRN2); x ships as bf16
    to halve the startup HBM load. K/Q/V column blocks are produced inside the
    first segment, borrowing idle engine time under the exp stream; attn@V
    matmuls lag the exp stream (PSUM adds commute) so segment epilogues drain
    psO without stalling the PE FIFO.
"""

import ml_dtypes
import numpy as np

HEADS, DH, G, EPS = 4, 32, 8, 1e-5
B, C, HW = 4, 256, 64
N = HW * HW          # 4096 spatial positions (keys)
NQ = N // 2          # 2048 queries per core
NKB = N // 128       # 32 key blocks
NQB = NQ // 512      # 4 query blocks of 512
CT = C // 128        # 2 channel tiles
HID = HEADS * DH     # 128

_BUILT = {}

# tuning knobs (read at build time)
CFG = {
    "flush_ci": 10,     # tile index where previous segment epilogue flushes
    "np": 18,           # ScalarE exp pairs per steady pair-segment
    "np0": 24,          # ScalarE exp pairs in segment 0
    "psd_bufs": 3,      # psD pool buffers (psd_bufs + pso_bufs == 4)
    "pso_bufs": 1,      # psO pool buffers
    "prefetch": False,  # pre-attention K1/V0/V1 via ScalarE copies
    "tail_split": False,  # final segment: half-granular yt + output DMA
    "tail_lag": 26,     # tiles between qb transposes and out-projection MMs
    "yt_lag": 30,       # tiles until the bias-add + output DMA
    "yp_psS": False,    # out-projection psum via the ScalarE-side pool
    "bn_pool": False,   # GroupNorm bn_stats on GpSimd (SBUF-only op)
    "oacc_hi": 20,      # attn@V emitter queue high-water mark
    "fused_epilogue": True,  # per-qsub epilogue/out-projection pipelining
    "xchunks": 8,       # DMA chunk count for the x load
    "ocp": None,        # None | "dve" | "act": copy psO->SBUF first
    "xn_pool": False,   # GroupNorm-apply on GpSimd instead of VectorE
    "ebufs": 13,        # exp-output tile double-buffering depth
    "epi_act": False,   # epilogue normalize-scale on ScalarE instead of DVE
}


_MAX_INST_WAITS = 1


def _patch_tail_drain(tile_mod):
    """Walrus codegen on this toolchain only supports a small number of sync
    waits per ISA instruction. Two patches:
    1. every committed instruction with too many waits gets the excess hoisted
       onto same-engine nops emitted immediately before it (same stream
       position => identical semantics);
    2. the TileContext tail drain (one wait per engine + DMA lane, >8 total)
       is split one wait per SP nop."""
    if getattr(tile_mod.TileContext, "_drain_patched", False):
        return
    import bass_rust
    from concourse.vector_clock import ScopedClock

    _orig_add_instruction = tile_mod.TileContext._add_instruction

    _SELF_WAIT_OK = ("InstActivation",)

    def _add_with_wait_split(self, inst):
        si = getattr(inst, "sync_info", None)
        if (
            si is not None
            and si.on_wait
            and type(inst).__name__ in _SELF_WAIT_OK
        ):
            # engine queues are strict FIFO: a data op's wait on its own
            # engine's sem is redundant (prior same-engine ops complete in
            # order before it) -- dropping it avoids a split nop per op
            eng_name = str(inst.engine).split(".")[-1]
            kept = [
                w for w in si.on_wait
                if w.ant_name.rsplit("_", 1)[0] != eng_name
            ]
            if len(kept) != len(si.on_wait):
                import bass_rust as _br
                inst.sync_info = _br.SyncInfo(
                    on_wait=kept, on_update=list(si.on_update)
                )
                si = inst.sync_info
        if si is not None and len(si.on_wait) > _MAX_INST_WAITS:
            waits = list(si.on_wait)
            keep, excess = waits[: _MAX_INST_WAITS], waits[_MAX_INST_WAITS :]
            eng = self.nc.engines[inst.engine]
            for w in excess:  # NoOps only support a single wait slot
                nop = eng.nop(nofuse=True, hint="wait_split")
                nop.ins.sync_info = bass_rust.SyncInfo(on_wait=[w], on_update=[])
            inst.sync_info = bass_rust.SyncInfo(
                on_wait=keep, on_update=list(si.on_update)
            )
        return _orig_add_instruction(self, inst)

    tile_mod.TileContext._add_instruction = _add_with_wait_split

    def _patched(self, tick_clock, wait_clock):
        nop = self.nc.sync.nop(nofuse=True, hint="pre_drain_wait_split")
        wait_clock.add_sem_waits(nop.ins, ScopedClock({None: tick_clock.global_clock}))
        si = nop.ins.sync_info
        waits = list(si.on_wait) if si is not None else []
        if len(waits) > 1:
            nop.ins.sync_info = bass_rust.SyncInfo(
                on_wait=waits[:1], on_update=list(si.on_update)
            )
            for i in range(1, len(waits)):
                n2 = self.nc.sync.nop(nofuse=True, hint="pre_drain_wait_split")
                n2.ins.sync_info = bass_rust.SyncInfo(on_wait=[waits[i]], on_update=[])
        self.nc.sync.drain()
        self.nc.all_engine_barrier()
        assert self.sems is not None
        popped = self.nc._tile_sem_poison_stack.pop()
        assert popped is self._sem_poison
        self.nc.clear_and_free_semaphores(list(self.sems.allocated().values()))
        # no trailing all_engine_barrier: NEFF completion already requires the
        # SP stream (which performs the sem clears) to finish, and every other
        # engine is quiesced by the barrier above

    tile_mod.TileContext._drain_and_barrier = _patched
    tile_mod.TileContext._drain_patched = True


def build_nc():
    import concourse.bass as bass
    import concourse.mybir as mybir
    import concourse.tile as tile

    _patch_tail_drain(tile)
    f32 = mybir.dt.float32
    bf16 = mybir.dt.bfloat16
    AF = mybir.ActivationFunctionType
    ALU = mybir.AluOpType

    nc = bass.Bass()
    x_d = nc.declare_dram_parameter("x", [CT, 128, N], bf16, isOutput=False)
    wq_d = nc.declare_dram_parameter("wq", [CT, 128, 3 * HID], f32, isOutput=False)
    wo_d = nc.declare_dram_parameter("wo", [HID, C], f32, isOutput=False)
    gb_d = nc.declare_dram_parameter("gb", [128, CT, 3], f32, isOutput=False)
    gmask_d = nc.declare_dram_parameter("gmask", [128, CT, G], f32, isOutput=False)
    sel_d = nc.declare_dram_parameter("selT", [G, CT, 128], f32, isOutput=False)
    out_d = nc.declare_dram_parameter("out", [CT, 128, NQ], bf16, isOutput=True)

    with tile.TileContext(nc) as tc:
        with (
            tc.tile_pool(name="big", bufs=1) as big,
            tc.tile_pool(name="work", bufs=2) as work,
            tc.tile_pool(name="epool", bufs=CFG["ebufs"]) as epool,
            tc.tile_pool(name="e2pool", bufs=CFG["ebufs"]) as e2pool,
            tc.tile_pool(name="hpool", bufs=2) as hpool,
            tc.tile_pool(name="psS", bufs=2, space="PSUM") as psS,  # 2 x 2 banks
            tc.tile_pool(name="psD", bufs=CFG["psd_bufs"], space="PSUM") as psD,
            tc.tile_pool(name="psO", bufs=CFG["pso_bufs"], space="PSUM") as psO,
        ):
            # ---------------- load inputs ----------------
            x_sb = big.tile([128, CT, N], bf16)
            nxc = CFG["xchunks"] // CT
            xw = N // nxc
            for t in range(CT):
                for j in range(nxc):
                    nc.sync.dma_start(
                        x_sb[:, t, j * xw : (j + 1) * xw],
                        x_d[t, :, j * xw : (j + 1) * xw],
                    )
            wq_sb = big.tile([128, CT, 3 * HID], f32)
            nc.sync.dma_start(wq_sb[:, 0], wq_d[0])
            nc.sync.dma_start(wq_sb[:, 1], wq_d[1])
            wo_sb = big.tile([HID, C], f32)
            nc.sync.dma_start(wo_sb[:], wo_d[:])
            gb_sb = big.tile([128, CT, 3], f32)
            nc.sync.dma_start(gb_sb[:], gb_d[:])
            gmask_sb = big.tile([128, CT, G], f32)
            nc.sync.dma_start(gmask_sb[:], gmask_d[:])
            sel_sb = big.tile([G, CT, 128], f32)
            nc.sync.dma_start(sel_sb[:], sel_d[:])

            wq_bf = big.tile([128, CT, 3 * HID], bf16)
            nc.scalar.activation(wq_bf[:], wq_sb[:], AF.Identity)
            wo_bf = big.tile([HID, C], bf16)
            nc.scalar.activation(wo_bf[:], wo_sb[:], AF.Identity)

            # ---------------- group norm ----------------
            # per-channel mean/var via DVE bn_stats (512-col chunks, overlaps
            # the x DMAs); T = [mean_c, E[x^2]_c]
            # Per-channel sums: every third 512-col chunk goes to the (idle)
            # ScalarE as Square/Identity passes with accum_out; the rest stay
            # on VectorE bn_stats. T[:,t] = [sum x, sum x^2] per channel
            # (later normalized by 1/(32*4096) inside the group mask weights).
            T_sb = work.tile([128, CT, 2], f32, tag="gnT")
            asc = work.tile([128, 512], bf16, tag="asc")  # ACT scratch
            for t in range(CT):
                bst = work.tile([128, 8, nc.vector.BN_STATS_DIM], f32, tag="gnbst")
                acc = work.tile([128, 4, 2], f32, tag="gnacc")
                na = 0
                for j in range(8):
                    sl = x_sb[:, t, j * 512 : (j + 1) * 512]
                    if j % 3 == 2:
                        nc.scalar.activation(
                            asc[:], sl, AF.Identity, accum_out=acc[:, na, 0:1]
                        )
                        nc.scalar.activation(
                            asc[:], sl, AF.Square, accum_out=acc[:, na, 1:2]
                        )
                        na += 1
                    else:
                        nc.vector.bn_stats(out=bst[:, j, :], in_=sl)
                # aggregate the VectorE chunks (6 of 8): bn_aggr needs a
                # contiguous bst block, so compact first
                nc.vector.tensor_copy(bst[:, 2, :], bst[:, 3, :])
                nc.vector.tensor_copy(bst[:, 3, :], bst[:, 4, :])
                nc.vector.tensor_copy(bst[:, 4, :], bst[:, 6, :])
                nc.vector.tensor_copy(bst[:, 5, :], bst[:, 7, :])
                mv = work.tile([128, nc.vector.BN_AGGR_DIM], f32, tag="gnmv")
                nc.vector.bn_aggr(out=mv[:], in_=bst[:, 0:6, :])
                # bn chunk sums: n1*mean, n1*(var + mean^2) with n1 = 6*512
                n1 = 6 * 512.0
                nc.vector.tensor_mul(T_sb[:, t, 1:2], mv[:, 0:1], mv[:, 0:1])
                nc.vector.tensor_tensor(
                    T_sb[:, t, 1:2], T_sb[:, t, 1:2], mv[:, 1:2], ALU.add
                )
                nc.vector.tensor_scalar(
                    T_sb[:, t, 1:2], T_sb[:, t, 1:2],
                    scalar1=n1, scalar2=None, op0=ALU.mult,
                )
                nc.vector.tensor_scalar(
                    T_sb[:, t, 0:1], mv[:, 0:1],
                    scalar1=n1, scalar2=None, op0=ALU.mult,
                )
                # add the ScalarE accumulator chunks
                for c in range(2):
                    nc.vector.tensor_tensor(
                        T_sb[:, t, c : c + 1], T_sb[:, t, c : c + 1],
                        acc[:, 0, c : c + 1], ALU.add,
                    )
                    nc.vector.tensor_tensor(
                        T_sb[:, t, c : c + 1], T_sb[:, t, c : c + 1],
                        acc[:, 1, c : c + 1], ALU.add,
                    )
            # group stats: [G, 2] = sum over channels-in-group / (32*4096)
            # (prologue borrows the psO banks before attention needs them)
            ps_pro = psO.tile([128, 512], f32, tag="oacc", name="pro")
            stats_ps = ps_pro[0:G, 0:2]
            for t in range(CT):
                nc.tensor.matmul(
                    stats_ps, gmask_sb[:, t, :], T_sb[:, t, :],
                    start=(t == 0), stop=(t == CT - 1),
                )
            # var = E[x^2] - mean^2 ; rstd = 1/sqrt(var+eps)
            stats_sb = work.tile([G, 2], f32, tag="gnstats")  # [mean, rstd]
            stats_tmp = work.tile([G, 2], f32, tag="gnstats_raw")
            msq = work.tile([G, 1], f32, tag="gnmsq")
            nc.vector.tensor_copy(stats_tmp[:], stats_ps)
            nc.vector.tensor_copy(stats_sb[:, 0:1], stats_tmp[:, 0:1])
            nc.vector.tensor_mul(msq[:], stats_tmp[:, 0:1], stats_tmp[:, 0:1])
            nc.vector.tensor_tensor(
                stats_sb[:, 1:2], stats_tmp[:, 1:2], msq[:], ALU.subtract
            )
            eps_sb = work.tile([G, 1], f32, tag="gneps")
            nc.vector.memset(eps_sb[:], EPS)
            nc.scalar.activation(
                stats_sb[:, 1:2], stats_sb[:, 1:2], AF.Sqrt, bias=eps_sb[:]
            )
            nc.vector.reciprocal(stats_sb[:, 1:2], stats_sb[:, 1:2])
            # broadcast to channels; gamma/beta fold into the projections
            scs = []
            bc_ps = []
            for t in range(CT):
                bc = ps_pro[:, 64 + 2 * t : 66 + 2 * t]  # [mean_c, rstd_c]
                nc.tensor.matmul(bc, sel_sb[:, t, :], stats_sb[:], start=True, stop=True)
                bc_ps.append(bc)
            for t in range(CT):
                sc = work.tile([128, 2], f32, tag=f"gnsc{t}")  # [scale_c, bias_c]
                nc.vector.tensor_mul(sc[:, 0:1], gb_sb[:, t, 0:1], bc_ps[t][:, 1:2])
                nc.vector.tensor_mul(sc[:, 1:2], bc_ps[t][:, 0:1], sc[:, 0:1])
                nc.vector.tensor_tensor(
                    sc[:, 1:2], gb_sb[:, t, 1:2], sc[:, 1:2], ALU.subtract
                )
                scs.append(sc)

            # ---------------- QKV projections ----------------
            # GroupNorm is folded into the projections: the per-channel scale
            # multiplies the projection weights (wq_sc), the per-channel bias
            # becomes a per-hid constant added during the psum->SBUF copies
            # (K/Q) or via a 1-row accumulating matmul (V^T). xn itself is
            # never materialized.
            # Only K block 0 and Q block 0 are produced up front; the rest is
            # interleaved into the first attention segment so ScalarE starts
            # exp-ing as early as possible.
            k_bf = big.tile([128, N], bf16)      # [hid, keys] head-major
            q_bf = big.tile([128, NQ], bf16)     # [hid, queries] (pre-scaled W)
            v_bf = big.tile([128, NKB, HID], bf16)  # [key128, kb, hid] via XBAR transpose
            vh_bf = big.tile([128, N], bf16)        # [hid, keys] V projection
            wq_sc = big.tile([128, CT, 3 * HID], bf16)  # weights x GN scale
            kq_bias = big.tile([128, 3], f32)   # [beta_K | beta_Q | beta_V]
            # K part first: the critical path to the first sim runs through
            # wq_sc[K] -> K0 -> sims; Q/V parts follow
            for p0 in (HID, 0, 2 * HID):
                for t in range(CT):
                    with nc.allow_low_precision(
                        reason="single rounding: f32 weights x GN scale -> bf16"
                    ):
                        nc.vector.tensor_scalar(
                            wq_sc[:, t, p0 : p0 + HID],
                            wq_sb[:, t, p0 : p0 + HID],
                            scalar1=scs[t][:, 0:1], scalar2=None, op0=ALU.mult,
                        )
            # beta_P[hid] = sum_c W_P[c, hid] * gnbias_c via PE; K/Q as [hid,1]
            # columns (per-partition add in the psum->SBUF copy), V as a
            # [1,hid] row (rank-1 accumulating matmul into V^T)
            bvec = work.tile([128, CT], bf16, tag="bvec")
            for t in range(CT):
                nc.vector.tensor_copy(bvec[:, t : t + 1], scs[t][:, 1:2])
            pb_ps = psD.tile([128, 512], f32, tag="simd", name="pbias")
            # col 0: beta_K, col 1: beta_Q, col 2: beta_V (per-hid columns)
            for p, w0 in enumerate([HID, 0, 2 * HID]):
                for t in range(CT):
                    nc.tensor.matmul(
                        pb_ps[:, p : p + 1],
                        wq_bf[:, t, w0 : w0 + HID],
                        bvec[:, t : t + 1],
                        start=(t == 0), stop=(t == CT - 1),
                    )
            nc.vector.tensor_copy(kq_bias[:], pb_ps[:, 0:3])
            ones1 = big.tile([128, 1], bf16)
            nc.vector.memset(ones1[:], 1.0)

            def emit_k(cb):
                kp = psD.tile([128, 512], f32, tag="simd", name=f"kp{cb}")[:, :]
                for t in range(CT):
                    nc.tensor.matmul(
                        kp, wq_sc[:, t, HID : 2 * HID],
                        x_sb[:, t, cb * 512 : (cb + 1) * 512],
                        start=(t == 0), stop=(t == CT - 1),
                    )
                if cb == 0:  # ScalarE is idle pre-attention; keep DVE free.
                    # kb=0's columns land first so tile 0's sims start early
                    nc.scalar.activation(
                        k_bf[:, 0:128], kp[:, 0:128], AF.Identity,
                        bias=kq_bias[:, 0:1],
                    )
                    nc.scalar.activation(
                        k_bf[:, 128:512], kp[:, 128:512], AF.Identity,
                        bias=kq_bias[:, 0:1],
                    )
                else:
                    nc.vector.tensor_scalar(
                        k_bf[:, cb * 512 : (cb + 1) * 512], kp,
                        scalar1=kq_bias[:, 0:1], scalar2=None, op0=ALU.add,
                    )

            def emit_q(cb):
                qp = psD.tile([128, 512], f32, tag="simd", name=f"qp{cb}")[:, :]
                for t in range(CT):
                    nc.tensor.matmul(
                        qp, wq_sc[:, t, 0:HID],
                        x_sb[:, t, cb * 512 : (cb + 1) * 512],
                        start=(t == 0), stop=(t == CT - 1),
                    )
                if cb == 0:
                    nc.scalar.activation(
                        q_bf[:, 0:512], qp, AF.Identity, bias=kq_bias[:, 1:2]
                    )
                else:
                    nc.vector.tensor_scalar(
                        q_bf[:, cb * 512 : (cb + 1) * 512], qp,
                        scalar1=kq_bias[:, 1:2], scalar2=None, op0=ALU.add,
                    )

            def emit_v(cb, use_act=False):
                # V as [hid, keys] (same shape as K), bias folded into the
                # psum->SBUF copy, then per-128 XBAR transposes into v_bf
                vp = psD.tile([128, 512], f32, tag="simd", name=f"vp{cb}")[:, :]
                for t in range(CT):
                    nc.tensor.matmul(
                        vp, wq_sc[:, t, 2 * HID : 3 * HID],
                        x_sb[:, t, cb * 512 : (cb + 1) * 512],
                        start=(t == 0), stop=(t == CT - 1),
                    )
                c_sl = slice(cb * 512, (cb + 1) * 512)
                if use_act:
                    nc.scalar.activation(
                        vh_bf[:, c_sl], vp, AF.Identity, bias=kq_bias[:, 2:3]
                    )
                else:
                    nc.vector.tensor_scalar(
                        vh_bf[:, c_sl], vp,
                        scalar1=kq_bias[:, 2:3], scalar2=None, op0=ALU.add,
                    )
                for kb in range(4 * cb, 4 * cb + 4):
                    nc.sync.dma_start_transpose(
                        v_bf[:, kb, :], vh_bf[:, kb * 128 : (kb + 1) * 128]
                    )
            zero_sb = big.tile([1, 512], bf16)
            nc.vector.memset(zero_sb[:], 0.0)
            emit_k(0)
            emit_q(0)


            # ---------------- attention + out-projection ----------------
            # Segment = (query block of 512, head pair): 64 slices (kb = i//2,
            # head = 2*pair + i%2), each slice one [128 keys x 512 q] sim
            # plane. Sim tiles hold 3 (or 2) planes in the 6-bank psA pool and
            # are consumed whole either by ScalarE (exact exp) or VectorE
            # (one-op Schraudolph exp in bf16 bit space, int16 out) -- bigger
            # tiles amortize each engine's fixed access cost. attn@V: per
            # slice 4 matmuls (one per 128-query block), exp weights
            # stationary, V^T_ext (32 V + 1 ones col) moving:
            # psO[q, 66*qs+33*hi] += e[:, qs]^T @ v_ext accumulated over kb;
            # one pair's worth fits a single PSUM bank.
            SLICES = NKB * 2  # 64 per segment

            def seg_pattern(npair):
                # npair 2-slice ScalarE exp tiles (psS) interleaved with
                # 1-slice VectorE Schraudolph tiles (psD); each stream is
                # independently double-buffered. Head tiles feed ScalarE (the
                # epilogue owns VectorE's stream head); tail tiles feed
                # VectorE so the next epilogue is not gated on ScalarE's lag.
                nd = SLICES - 2 * npair
                pat, a, d = ["A", "A"], 2, 0
                while a < npair or d < nd - 2:
                    if (d * npair >= a * nd and a < npair) or d >= nd - 2:
                        pat.append("A")
                        a += 1
                    else:
                        pat.append("D")
                        d += 1
                pat += ["D", "D"]
                return pat

            def make_epilogue(qb, pair, oacc, hT):
                # per qsub: 1/denominator (on the query's own partition),
                # broadcast-multiply O^T into this pair's half of hT
                def _emit(zmm):
                    ov = oacc[:, 0 : 4 * 66].rearrange("p (q c) -> p q c", c=66)
                    rcp = work.tile([128, 4, 2], f32, tag="rcp")
                    with nc.allow_low_precision(
                        reason="softmax denom reciprocal feeds bf16 output"
                    ):
                        nc.vector.reciprocal(rcp[:], ov[:, :, 64:66])
                        nc.vector.tensor_tensor(
                            hT[:, :, 64 * pair : 64 * pair + 64].rearrange(
                                "p q (h c) -> p q h c", c=DH
                            ),
                            ov[:, :, 0:64].rearrange(
                                "p q (h c) -> p q h c", c=DH
                            ),
                            rcp[:].unsqueeze(3).broadcast_to([128, 4, 2, DH]),
                            ALU.mult,
                        )
                    if zmm is not None:
                        zmm()
                return _emit

            def make_qb_transp(qb, hT):
                def _emit(zmm):
                    h_sb = hpool.tile([128, 512], bf16, tag="h")
                    for qs in range(4):
                        nc.sync.dma_start_transpose(
                            h_sb[:, qs * 128 : (qs + 1) * 128], hT[:, qs]
                        )
                    return h_sb
                return _emit

            def make_qb_mms(qb, h_box, yp_box):
                # out-projection over the transposed h (emitted a few tiles
                # after the transposes so their DMA latency stays off the PE
                # FIFO's critical path)
                def _emit(zmm):
                    h_sb = h_box[0]
                    if CFG["yp_psS"]:
                        ypt = psS.tile([128, 2, 512], f32, tag="sim", name=f"yp{qb}")
                        yps = [ypt[:, oc, :] for oc in range(CT)]
                    else:
                        yps = [
                            psD.tile([128, 512], f32, tag="simd", name=f"yp{qb}_{oc}")[:, :]
                            for oc in range(CT)
                        ]
                    yp_box.append(yps)
                    for qs in range(4):
                        for oc in range(CT):
                            nc.tensor.matmul(
                                yps[oc][:, qs * 128 : (qs + 1) * 128],
                                wo_bf[:, oc * 128 : (oc + 1) * 128],
                                h_sb[:, qs * 128 : (qs + 1) * 128],
                                start=True, stop=True,
                            )
                return _emit

            def make_qb_tail(qb, yp_box, use_act):
                def _emit(zmm):  # zmm unused: only the epilogue re-zeroes psO
                    yps = yp_box[0]
                    q0 = qb * 512
                    if use_act and CFG["tail_split"]:
                        # tail: ScalarE is idle after the last exp; halves let
                        # the output DMA start while qs2/3 are still in flight
                        yts = [
                            work.tile([128, 512], bf16, tag=f"yt{oc}", name=f"yt{oc}")
                            for oc in range(CT)
                        ]
                        for half in range(2):
                            c_sl = slice(half * 256, half * 256 + 256)
                            for oc in range(CT):
                                nc.scalar.activation(
                                    yts[oc][:, c_sl], yps[oc][:, c_sl],
                                    AF.Identity, bias=gb_sb[:, oc, 2:3],
                                )
                                nc.sync.dma_start(
                                    out_d[oc, :, q0 + half * 256 : q0 + half * 256 + 256],
                                    yts[oc][:, c_sl],
                                )
                    else:
                        for oc in range(CT):
                            yt = work.tile([128, 512], bf16, tag="yt")
                            if use_act:
                                nc.scalar.activation(
                                    yt[:], yps[oc][:], AF.Identity,
                                    bias=gb_sb[:, oc, 2:3],
                                )
                            else:
                                nc.vector.tensor_scalar(
                                    yt[:], yps[oc][:], scalar1=gb_sb[:, oc, 2:3],
                                    scalar2=None, op0=ALU.add,
                                )
                            nc.sync.dma_start(
                                out_d[oc, :, q0 : q0 + 512], yt[:]
                            )
                return _emit

            pending = []    # deferred epilogue / qb-tail emitters
            oaccq = []      # attn@V emitters lagging the exp stream

            def emit_sim(s_idx, tile_ap, plane, qb, pair):
                kb, hi = s_idx // 2, s_idx % 2
                h = 2 * pair + hi
                q_sl = slice(qb * 512, (qb + 1) * 512)
                nc.tensor.matmul(
                    tile_ap[:, plane, :],
                    k_bf[32 * h : 32 * h + 32, kb * 128 : (kb + 1) * 128],
                    q_bf[32 * h : 32 * h + 32, q_sl],
                    start=True, stop=True,
                    tile_position=(32 * h, 0),
                )

            def make_oacc(s_idx, e_ap, oacc, pair):
                kb, hi = s_idx // 2, s_idx % 2
                h = 2 * pair + hi
                def _emit(last=False):
                    # PSUM adds commute, so attn@V order across slices is
                    # free; stop (clearing the bank's group-started state)
                    # must ride the final matmul actually emitted
                    for qs in range(4):
                        col = 66 * qs + 32 * hi
                        nc.tensor.matmul(
                            oacc[:, col : col + 32],
                            e_ap[:, qs * 128 : (qs + 1) * 128],
                            v_bf[:, kb, 32 * h : 32 * h + 32],
                            start=False, stop=False,
                        )
                        nc.tensor.matmul(
                            oacc[:, 66 * qs + 64 + hi : 66 * qs + 65 + hi],
                            e_ap[:, qs * 128 : (qs + 1) * 128],
                            ones1[:],
                            start=False,
                            stop=(last and qs == 3),
                        )
                return _emit

            hT = None
            for sg in range(2 * NQB):
                qb, pair = sg // 2, sg % 2
                seg0 = sg == 0
                oacc = psO.tile([128, 512], f32, tag="oacc", name=f"o{sg}")

                def zmm(oacc=oacc):
                    # one start=True matmul filling the whole bank resets its
                    # pending-zero state in one shot, so the interleaved
                    # per-(head, qsub) groups can then pure-accumulate (a
                    # start per group would wipe the others' first block)
                    nc.tensor.matmul(
                        oacc[:], zero_sb[:, 0:128], zero_sb[:],
                        start=True, stop=False,
                    )

                if seg0:
                    zmm()
                if pair == 0:
                    hT = hpool.tile([128, 4, HID], bf16, tag="hT")
                pat = seg_pattern(CFG["np0"] if seg0 else CFG["np"])
                i = 0
                for ci, kind in enumerate(pat):
                    ts_ = 2 if kind == "A" else 1
                    if kind == "A":
                        tile = psS.tile([128, 2, 512], f32, tag="sim", name="sim")
                    else:
                        tile = psD.tile([128, 512], f32, tag="simd", name="simd").unsqueeze(1)
                    for s in range(ts_):
                        emit_sim(i + s, tile, s, qb, pair)
                    due = [
                        (o, fn) for o, fn in pending
                        if ci >= CFG["flush_ci"] + o
                    ]
                    if due:
                        # previous segment's epilogue: its DVE ops lead the
                        # queue; the freed psO bank is re-zeroed (zmm) right
                        # after its last read retires
                        for o, fn in due:
                            fn(zmm if not seg0 else None)
                        pending = [p for p in pending if p not in due]
                    if seg0:
                        # produce the next V / K column blocks, overlapped
                        # with the exp stream (V leads: its transposes add
                        # DMA latency before the attn@V needs it)
                        for kb in range((i + 1) // 2, (i + ts_ + 1) // 2):
                            if kb % 4 == 0 and 0 <= kb // 4 < 8:
                                emit_v(kb // 4)
                            if kb % 4 == 2 and 0 < kb // 4 + 1 < 8:
                                emit_k(kb // 4 + 1)
                    if qb < NQB - 1 and pair == 1 and ci == 10:
                        emit_q(qb + 1)
                    if kind == "A":
                        e = epool.tile([128, 2, 512], bf16, tag="e")
                        nc.scalar.activation(e[:], tile[:], AF.Exp)
                        for s in range(2):
                            oaccq.append(make_oacc(i + s, e[:, s, :], oacc, pair))
                    else:
                        e2 = e2pool.tile([128, 512], mybir.dt.int16, tag="e2")
                        with nc.allow_low_precision(
                            reason="Schraudolph bf16 exp on DVE; softmax renormalizes"
                        ):
                            nc.vector.tensor_scalar(
                                e2[:], tile[:, 0, :],
                                scalar1=128.0 / 0.6931471805599453,
                                scalar2=(127.0 - 0.043) * 128.0,
                                op0=ALU.mult, op1=ALU.add,
                            )
                        oaccq.append(make_oacc(i, e2.bitcast(bf16), oacc, pair))
                    i += ts_
                    while len(oaccq) > CFG["oacc_hi"]:
                        oaccq.pop(0)()
                while oaccq:
                    fn_, last_ = oaccq.pop(0), not oaccq
                    fn_(last_)
                pending.append((0, make_epilogue(qb, pair, oacc, hT)))
                if pair == 1:
                    h_box, yp_box = [], []
                    tp = make_qb_transp(qb, hT)
                    pending.append(
                        (0, lambda z, tp=tp, h_box=h_box: h_box.append(tp(None)))
                    )
                    pending.append((CFG["tail_lag"], make_qb_mms(qb, h_box, yp_box)))
                    pending.append(
                        (CFG["yt_lag"], make_qb_tail(qb, yp_box, use_act=(qb == NQB - 1)))
                    )
            for _, fn in pending:
                fn(None)
    return nc


def _prep_shared(w_qkv, w_out, b_out, gamma, beta):
    scale = DH ** -0.5
    wqkvT = np.ascontiguousarray(w_qkv.T).astype(np.float32).copy()  # [C, 384]
    wqkvT[:, :HID] *= scale
    wq = np.ascontiguousarray(wqkvT.reshape(CT, 128, 3 * HID))
    wo = np.ascontiguousarray(w_out.T).astype(np.float32)            # [HID, C]
    gb = np.stack(
        [
            np.asarray(gamma, np.float32).reshape(CT, 128).T,
            np.asarray(beta, np.float32).reshape(CT, 128).T,
            np.asarray(b_out, np.float32).reshape(CT, 128).T,
        ],
        axis=-1,
    )  # [128, CT, 3]
    gmask = np.zeros((128, CT, G), np.float32)
    sel = np.zeros((G, CT, 128), np.float32)
    for t in range(CT):
        for p in range(128):
            g = (t * 128 + p) // (C // G)
            gmask[p, t, g] = 1.0 / ((C // G) * N)
            sel[g, t, p] = 1.0
    return wq, wo, gb, gmask, sel


def _run(inputs, trace=False):
    from concourse.bass_utils import run_bass_kernel_spmd

    x = np.asarray(inputs["x"], np.float32)
    wq, wo, gb, gmask, sel = _prep_shared(
        np.asarray(inputs["w_qkv"], np.float32),
        np.asarray(inputs["w_out"], np.float32),
        np.asarray(inputs["b_out"], np.float32),
        np.asarray(inputs["gamma"], np.float32),
        np.asarray(inputs["beta"], np.float32),
    )
    if "nc" not in _BUILT:
        _BUILT["nc"] = build_nc()
    nc = _BUILT["nc"]

    in_maps = []
    for core in range(8):
        b_idx, qh = core // 2, core % 2
        xb = x[b_idx].reshape(C, N)
        if qh:
            xb = np.roll(xb, -NQ, axis=1)
        in_maps.append(
            {
                "x": np.ascontiguousarray(
                    xb.reshape(CT, 128, N).astype(ml_dtypes.bfloat16)
                ),
                "wq": wq, "wo": wo, "gb": gb, "gmask": gmask, "selT": sel,
            }
        )
    res = run_bass_kernel_spmd(
        nc, in_maps, core_ids=list(range(8)), trace=trace
    )
    out = np.empty((B, C, N), np.float32)
    for core in range(8):
        b_idx, qh = core // 2, core % 2
        y = res.results[core]["out"].astype(np.float32).reshape(C, NQ)
        out[b_idx, :, qh * NQ : (qh + 1) * NQ] = y
    return out.reshape(B, C, HW, HW), res


def kernel(**inputs) -> np.ndarray:
    out, _ = _run(inputs, trace=False)
    return out


# revision 48
# speedup vs baseline: 1.0159x; 1.0033x over previous
"""Trainium2 Bass kernel: GroupNorm(8) -> 1x1 QKV conv -> 4-head attention
(n=4096, dim_head=32) -> 1x1 out conv, for x[4, 256, 64, 64] f32.

Sharding (8 cores, SPMD, no collectives): core c handles batch c//2 and query
half c%2. Each core receives the full 256x4096 (channels x spatial) slab of
its batch -- spatially ROLLED by 2048 for odd cores, so the program always
computes attention outputs for "queries 0:2048". GroupNorm statistics and
attention are invariant under a permutation of the key/spatial axis, so the
rolled copy yields exactly the outputs of the core's query half.

Device-side layout highlights:
  - GroupNorm is folded into the QKV projections: per-channel scale multiplies
    the (f32) projection weights once on-device; per-channel bias becomes a
    per-hid constant added during the psum->SBUF copies (K/Q/V). The
    normalized activation tensor is never materialized. Channel statistics
    come from VectorE bn_stats chunks plus ScalarE Square/Identity accum_out
    passes (ScalarE is idle while x streams in).
  - sim is computed transposed, [keys, queries], per head at PE row-strip 32h
    (tile_position row tiling). Work is organized as 8 pair-segments
    (query-block x head-pair) of 64 slices; each slice is a [128 keys x 512 q]
    sim plane. Exp is split between ScalarE (2-slice psS tiles, exact exp) and
    VectorE (1-slice psD tiles, one-op Schraudolph exp in bf16 bit space);
    both streams are independently double/triple buffered.
  - attn@V runs with the exp weights as the STATIONARY operand (128-query
    column slices) and V^T as the 32-wide MOVING operand (plus a 1-column
    ones matmul for the softmax denominator): psO[q, 66*qs+32*hi] += e^T @ v,
    33 PE column-cycles per slice-quarter instead of 512. One pair-segment
    fits a single PSUM bank; a single full-bank zeroing matmul per segment
    resets the pending-zero state so the interleaved per-(head, qsub)
    accumulation groups can pure-accumulate.
  - softmax normalization is one strided reciprocal + one broadcast multiply
    per segment (denominators land on their query's partition); h^T -> h goes
    through the DMA XBAR transpose (dma_start_transpose), costing no
    compute-engine time. V^T is likewise produced by XBAR transposes of a
    hid-major V projection. Out-projection matmuls are deferred ~26 tiles so
    transpose DMA latency never blocks the PE FIFO.
  - all matmuls are bf16 (fp32 matmul is 4x slower on TRN2); x ships as bf16
    to halve the startup HBM load. K/Q/V column blocks are produced inside the
    first segment, borrowing idle engine time under the exp stream; attn@V
    matmuls lag the exp stream (PSUM adds commute) so segment epilogues drain
    psO without stalling the PE FIFO.
"""

import ml_dtypes
import numpy as np

HEADS, DH, G, EPS = 4, 32, 8, 1e-5
B, C, HW = 4, 256, 64
N = HW * HW          # 4096 spatial positions (keys)
NQ = N // 2          # 2048 queries per core
NKB = N // 128       # 32 key blocks
NQB = NQ // 512      # 4 query blocks of 512
CT = C // 128        # 2 channel tiles
HID = HEADS * DH     # 128

_BUILT = {}

# tuning knobs (read at build time)
CFG = {
    "flush_ci": 11,     # tile index where previous segment epilogue flushes
    "np": 18,           # ScalarE exp pairs per steady pair-segment
    "np0": 24,          # ScalarE exp pairs in segment 0
    "psd_bufs": 3,      # psD pool buffers (psd_bufs + pso_bufs == 4)
    "pso_bufs": 1,      # psO pool buffers
    "prefetch": False,  # pre-attention K1/V0/V1 via ScalarE copies
    "tail_split": False,  # final segment: half-granular yt + output DMA
    "tail_lag": 26,     # tiles between qb transposes and out-projection MMs
    "yt_lag": 30,       # tiles until the bias-add + output DMA
    "yp_psS": False,    # out-projection psum via the ScalarE-side pool
    "kact": 0,          # K blocks (beyond 0) copied via ScalarE
    "vact": 2,          # V blocks copied via ScalarE
    "bn_pool": False,   # GroupNorm bn_stats on GpSimd (SBUF-only op)
    "oacc_hi": 22,      # attn@V emitter queue high-water mark
    "fused_epilogue": True,  # per-qsub epilogue/out-projection pipelining
    "xchunks": 8,       # DMA chunk count for the x load
    "ocp": None,        # None | "dve" | "act": copy psO->SBUF first
    "xn_pool": False,   # GroupNorm-apply on GpSimd instead of VectorE
    "ebufs": 13,        # exp-output tile double-buffering depth
    "epi_act": False,   # epilogue normalize-scale on ScalarE instead of DVE
}


_MAX_INST_WAITS = 1


def _patch_tail_drain(tile_mod):
    """Walrus codegen on this toolchain only supports a small number of sync
    waits per ISA instruction. Two patches:
    1. every committed instruction with too many waits gets the excess hoisted
       onto same-engine nops emitted immediately before it (same stream
       position => identical semantics);
    2. the TileContext tail drain (one wait per engine + DMA lane, >8 total)
       is split one wait per SP nop."""
    if getattr(tile_mod.TileContext, "_drain_patched", False):
        return
    import bass_rust
    from concourse.vector_clock import ScopedClock

    _orig_add_instruction = tile_mod.TileContext._add_instruction

    _SELF_WAIT_OK = ("InstActivation",)

    def _add_with_wait_split(self, inst):
        si = getattr(inst, "sync_info", None)
        if (
            si is not None
            and si.on_wait
            and type(inst).__name__ in _SELF_WAIT_OK
        ):
            # engine queues are strict FIFO: a data op's wait on its own
            # engine's sem is redundant (prior same-engine ops complete in
            # order before it) -- dropping it avoids a split nop per op
            eng_name = str(inst.engine).split(".")[-1]
            kept = [
                w for w in si.on_wait
                if w.ant_name.rsplit("_", 1)[0] != eng_name
            ]
            if len(kept) != len(si.on_wait):
                import bass_rust as _br
                inst.sync_info = _br.SyncInfo(
                    on_wait=kept, on_update=list(si.on_update)
                )
                si = inst.sync_info
        if si is not None and len(si.on_wait) > _MAX_INST_WAITS:
            waits = list(si.on_wait)
            keep, excess = waits[: _MAX_INST_WAITS], waits[_MAX_INST_WAITS :]
            eng = self.nc.engines[inst.engine]
            for w in excess:  # NoOps only support a single wait slot
                nop = eng.nop(nofuse=True, hint="wait_split")
                nop.ins.sync_info = bass_rust.SyncInfo(on_wait=[w], on_update=[])
            inst.sync_info = bass_rust.SyncInfo(
                on_wait=keep, on_update=list(si.on_update)
            )
        return _orig_add_instruction(self, inst)

    tile_mod.TileContext._add_instruction = _add_with_wait_split

    def _patched(self, tick_clock, wait_clock):
        nop = self.nc.sync.nop(nofuse=True, hint="pre_drain_wait_split")
        wait_clock.add_sem_waits(nop.ins, ScopedClock({None: tick_clock.global_clock}))
        si = nop.ins.sync_info
        waits = list(si.on_wait) if si is not None else []
        if len(waits) > 1:
            nop.ins.sync_info = bass_rust.SyncInfo(
                on_wait=waits[:1], on_update=list(si.on_update)
            )
            for i in range(1, len(waits)):
                n2 = self.nc.sync.nop(nofuse=True, hint="pre_drain_wait_split")
                n2.ins.sync_info = bass_rust.SyncInfo(on_wait=[waits[i]], on_update=[])
        self.nc.sync.drain()
        self.nc.all_engine_barrier()
        assert self.sems is not None
        popped = self.nc._tile_sem_poison_stack.pop()
        assert popped is self._sem_poison
        self.nc.clear_and_free_semaphores(list(self.sems.allocated().values()))
        # no trailing all_engine_barrier: NEFF completion already requires the
        # SP stream (which performs the sem clears) to finish, and every other
        # engine is quiesced by the barrier above

    tile_mod.TileContext._drain_and_barrier = _patched
    tile_mod.TileContext._drain_patched = True


def build_nc():
    import concourse.bass as bass
    import concourse.mybir as mybir
    import concourse.tile as tile

    _patch_tail_drain(tile)
    f32 = mybir.dt.float32
    bf16 = mybir.dt.bfloat16
    AF = mybir.ActivationFunctionType
    ALU = mybir.AluOpType

    nc = bass.Bass()
    x_d = nc.declare_dram_parameter("x", [CT, 128, N], bf16, isOutput=False)
    wq_d = nc.declare_dram_parameter("wq", [CT, 128, 3 * HID], f32, isOutput=False)
    wo_d = nc.declare_dram_parameter("wo", [HID, C], f32, isOutput=False)
    gb_d = nc.declare_dram_parameter("gb", [128, CT, 3], f32, isOutput=False)
    gmask_d = nc.declare_dram_parameter("gmask", [128, CT, G], f32, isOutput=False)
    sel_d = nc.declare_dram_parameter("selT", [G, CT, 128], f32, isOutput=False)
    out_d = nc.declare_dram_parameter("out", [CT, 128, NQ], bf16, isOutput=True)

    with tile.TileContext(nc) as tc:
        with (
            tc.tile_pool(name="big", bufs=1) as big,
            tc.tile_pool(name="work", bufs=2) as work,
            tc.tile_pool(name="epool", bufs=CFG["ebufs"]) as epool,
            tc.tile_pool(name="e2pool", bufs=CFG["ebufs"]) as e2pool,
            tc.tile_pool(name="hpool", bufs=2) as hpool,
            tc.tile_pool(name="psS", bufs=2, space="PSUM") as psS,  # 2 x 2 banks
            tc.tile_pool(name="psD", bufs=CFG["psd_bufs"], space="PSUM") as psD,
            tc.tile_pool(name="psO", bufs=CFG["pso_bufs"], space="PSUM") as psO,
        ):
            # ---------------- load inputs ----------------
            x_sb = big.tile([128, CT, N], bf16)
            nxc = CFG["xchunks"] // CT
            xw = N // nxc
            for t in range(CT):
                for j in range(nxc):
                    nc.sync.dma_start(
                        x_sb[:, t, j * xw : (j + 1) * xw],
                        x_d[t, :, j * xw : (j + 1) * xw],
                    )
            wq_sb = big.tile([128, CT, 3 * HID], f32)
            nc.sync.dma_start(wq_sb[:, 0], wq_d[0])
            nc.sync.dma_start(wq_sb[:, 1], wq_d[1])
            wo_sb = big.tile([HID, C], f32)
            nc.sync.dma_start(wo_sb[:], wo_d[:])
            gb_sb = big.tile([128, CT, 3], f32)
            nc.sync.dma_start(gb_sb[:], gb_d[:])
            gmask_sb = big.tile([128, CT, G], f32)
            nc.sync.dma_start(gmask_sb[:], gmask_d[:])
            sel_sb = big.tile([G, CT, 128], f32)
            nc.sync.dma_start(sel_sb[:], sel_d[:])

            wq_bf = big.tile([128, CT, 3 * HID], bf16)
            nc.scalar.activation(wq_bf[:], wq_sb[:], AF.Identity)
            wo_bf = big.tile([HID, C], bf16)
            nc.scalar.activation(wo_bf[:], wo_sb[:], AF.Identity)

            # ---------------- group norm ----------------
            # per-channel mean/var via DVE bn_stats (512-col chunks, overlaps
            # the x DMAs); T = [mean_c, E[x^2]_c]
            # Per-channel sums: every third 512-col chunk goes to the (idle)
            # ScalarE as Square/Identity passes with accum_out; the rest stay
            # on VectorE bn_stats. T[:,t] = [sum x, sum x^2] per channel
            # (later normalized by 1/(32*4096) inside the group mask weights).
            T_sb = work.tile([128, CT, 2], f32, tag="gnT")
            asc = work.tile([128, 512], bf16, tag="asc")  # ACT scratch
            for t in range(CT):
                bst = work.tile([128, 8, nc.vector.BN_STATS_DIM], f32, tag="gnbst")
                acc = work.tile([128, 4, 2], f32, tag="gnacc")
                na = 0
                for j in range(8):
                    sl = x_sb[:, t, j * 512 : (j + 1) * 512]
                    if j % 3 == 2:
                        nc.scalar.activation(
                            asc[:], sl, AF.Identity, accum_out=acc[:, na, 0:1]
                        )
                        nc.scalar.activation(
                            asc[:], sl, AF.Square, accum_out=acc[:, na, 1:2]
                        )
                        na += 1
                    else:
                        nc.vector.bn_stats(out=bst[:, j, :], in_=sl)
                # aggregate the VectorE chunks (6 of 8): bn_aggr needs a
                # contiguous bst block, so compact first
                nc.vector.tensor_copy(bst[:, 2, :], bst[:, 3, :])
                nc.vector.tensor_copy(bst[:, 3, :], bst[:, 4, :])
                nc.vector.tensor_copy(bst[:, 4, :], bst[:, 6, :])
                nc.vector.tensor_copy(bst[:, 5, :], bst[:, 7, :])
                mv = work.tile([128, nc.vector.BN_AGGR_DIM], f32, tag="gnmv")
                nc.vector.bn_aggr(out=mv[:], in_=bst[:, 0:6, :])
                # bn chunk sums: n1*mean, n1*(var + mean^2) with n1 = 6*512
                n1 = 6 * 512.0
                nc.vector.tensor_mul(T_sb[:, t, 1:2], mv[:, 0:1], mv[:, 0:1])
                nc.vector.tensor_tensor(
                    T_sb[:, t, 1:2], T_sb[:, t, 1:2], mv[:, 1:2], ALU.add
                )
                nc.vector.tensor_scalar(
                    T_sb[:, t, 1:2], T_sb[:, t, 1:2],
                    scalar1=n1, scalar2=None, op0=ALU.mult,
                )
                nc.vector.tensor_scalar(
                    T_sb[:, t, 0:1], mv[:, 0:1],
                    scalar1=n1, scalar2=None, op0=ALU.mult,
                )
                # add the ScalarE accumulator chunks
                for c in range(2):
                    nc.vector.tensor_tensor(
                        T_sb[:, t, c : c + 1], T_sb[:, t, c : c + 1],
                        acc[:, 0, c : c + 1], ALU.add,
                    )
                    nc.vector.tensor_tensor(
                        T_sb[:, t, c : c + 1], T_sb[:, t, c : c + 1],
                        acc[:, 1, c : c + 1], ALU.add,
                    )
            # group stats: [G, 2] = sum over channels-in-group / (32*4096)
            # (prologue borrows the psO banks before attention needs them)
            ps_pro = psO.tile([128, 512], f32, tag="oacc", name="pro")
            stats_ps = ps_pro[0:G, 0:2]
            for t in range(CT):
                nc.tensor.matmul(
                    stats_ps, gmask_sb[:, t, :], T_sb[:, t, :],
                    start=(t == 0), stop=(t == CT - 1),
                )
            # var = E[x^2] - mean^2 ; rstd = 1/sqrt(var+eps)
            stats_sb = work.tile([G, 2], f32, tag="gnstats")  # [mean, rstd]
            stats_tmp = work.tile([G, 2], f32, tag="gnstats_raw")
            msq = work.tile([G, 1], f32, tag="gnmsq")
            nc.vector.tensor_copy(stats_tmp[:], stats_ps)
            nc.vector.tensor_copy(stats_sb[:, 0:1], stats_tmp[:, 0:1])
            nc.vector.tensor_mul(msq[:], stats_tmp[:, 0:1], stats_tmp[:, 0:1])
            nc.vector.tensor_tensor(
                stats_sb[:, 1:2], stats_tmp[:, 1:2], msq[:], ALU.subtract
            )
            eps_sb = work.tile([G, 1], f32, tag="gneps")
            nc.vector.memset(eps_sb[:], EPS)
            nc.scalar.activation(
                stats_sb[:, 1:2], stats_sb[:, 1:2], AF.Sqrt, bias=eps_sb[:]
            )
            nc.vector.reciprocal(stats_sb[:, 1:2], stats_sb[:, 1:2])
            # broadcast to channels; gamma/beta fold into the projections
            scs = []
            bc_ps = []
            for t in range(CT):
                bc = ps_pro[:, 64 + 2 * t : 66 + 2 * t]  # [mean_c, rstd_c]
                nc.tensor.matmul(bc, sel_sb[:, t, :], stats_sb[:], start=True, stop=True)
                bc_ps.append(bc)
            for t in range(CT):
                sc = work.tile([128, 2], f32, tag=f"gnsc{t}")  # [scale_c, bias_c]
                nc.vector.tensor_mul(sc[:, 0:1], gb_sb[:, t, 0:1], bc_ps[t][:, 1:2])
                nc.vector.tensor_mul(sc[:, 1:2], bc_ps[t][:, 0:1], sc[:, 0:1])
                nc.vector.tensor_tensor(
                    sc[:, 1:2], gb_sb[:, t, 1:2], sc[:, 1:2], ALU.subtract
                )
                scs.append(sc)

            # ---------------- QKV projections ----------------
            # GroupNorm is folded into the projections: the per-channel scale
            # multiplies the projection weights (wq_sc), the per-channel bias
            # becomes a per-hid constant added during the psum->SBUF copies
            # (K/Q) or via a 1-row accumulating matmul (V^T). xn itself is
            # never materialized.
            # Only K block 0 and Q block 0 are produced up front; the rest is
            # interleaved into the first attention segment so ScalarE starts
            # exp-ing as early as possible.
            k_bf = big.tile([128, N], bf16)      # [hid, keys] head-major
            q_bf = big.tile([128, NQ], bf16)     # [hid, queries] (pre-scaled W)
            v_bf = big.tile([128, NKB, HID], bf16)  # [key128, kb, hid] via XBAR transpose
            vh_bf = big.tile([128, N], bf16)        # [hid, keys] V projection
            wq_sc = big.tile([128, CT, 3 * HID], bf16)  # weights x GN scale
            kq_bias = big.tile([128, 3], f32)   # [beta_K | beta_Q | beta_V]
            # K part first: the critical path to the first sim runs through
            # wq_sc[K] -> K0 -> sims; Q/V parts follow
            for p0 in (HID, 0, 2 * HID):
                for t in range(CT):
                    with nc.allow_low_precision(
                        reason="single rounding: f32 weights x GN scale -> bf16"
                    ):
                        nc.vector.tensor_scalar(
                            wq_sc[:, t, p0 : p0 + HID],
                            wq_sb[:, t, p0 : p0 + HID],
                            scalar1=scs[t][:, 0:1], scalar2=None, op0=ALU.mult,
                        )
            # beta_P[hid] = sum_c W_P[c, hid] * gnbias_c via PE; K/Q as [hid,1]
            # columns (per-partition add in the psum->SBUF copy), V as a
            # [1,hid] row (rank-1 accumulating matmul into V^T)
            bvec = work.tile([128, CT], bf16, tag="bvec")
            for t in range(CT):
                nc.vector.tensor_copy(bvec[:, t : t + 1], scs[t][:, 1:2])
            pb_ps = psD.tile([128, 512], f32, tag="simd", name="pbias")
            # col 0: beta_K, col 1: beta_Q, col 2: beta_V (per-hid columns)
            for p, w0 in enumerate([HID, 0, 2 * HID]):
                for t in range(CT):
                    nc.tensor.matmul(
                        pb_ps[:, p : p + 1],
                        wq_bf[:, t, w0 : w0 + HID],
                        bvec[:, t : t + 1],
                        start=(t == 0), stop=(t == CT - 1),
                    )
            nc.vector.tensor_copy(kq_bias[:], pb_ps[:, 0:3])
            ones1 = big.tile([128, 1], bf16)
            nc.vector.memset(ones1[:], 1.0)

            def emit_k(cb):
                kp = psD.tile([128, 512], f32, tag="simd", name=f"kp{cb}")[:, :]
                for t in range(CT):
                    nc.tensor.matmul(
                        kp, wq_sc[:, t, HID : 2 * HID],
                        x_sb[:, t, cb * 512 : (cb + 1) * 512],
                        start=(t == 0), stop=(t == CT - 1),
                    )
                if cb == 0:  # ScalarE is idle pre-attention; keep DVE free.
                    # kb=0's columns land first so tile 0's sims start early
                    nc.scalar.activation(
                        k_bf[:, 0:128], kp[:, 0:128], AF.Identity,
                        bias=kq_bias[:, 0:1],
                    )
                    nc.scalar.activation(
                        k_bf[:, 128:512], kp[:, 128:512], AF.Identity,
                        bias=kq_bias[:, 0:1],
                    )
                elif cb <= CFG["kact"]:
                    # early blocks: ScalarE stalls on these anyway during the
                    # seg0 ramp; a scaled copy beats the stall
                    nc.scalar.activation(
                        k_bf[:, cb * 512 : (cb + 1) * 512], kp, AF.Identity,
                        bias=kq_bias[:, 0:1],
                    )
                else:
                    nc.vector.tensor_scalar(
                        k_bf[:, cb * 512 : (cb + 1) * 512], kp,
                        scalar1=kq_bias[:, 0:1], scalar2=None, op0=ALU.add,
                    )

            def emit_q(cb):
                qp = psD.tile([128, 512], f32, tag="simd", name=f"qp{cb}")[:, :]
                for t in range(CT):
                    nc.tensor.matmul(
                        qp, wq_sc[:, t, 0:HID],
                        x_sb[:, t, cb * 512 : (cb + 1) * 512],
                        start=(t == 0), stop=(t == CT - 1),
                    )
                if cb == 0:
                    nc.scalar.activation(
                        q_bf[:, 0:512], qp, AF.Identity, bias=kq_bias[:, 1:2]
                    )
                else:
                    nc.vector.tensor_scalar(
                        q_bf[:, cb * 512 : (cb + 1) * 512], qp,
                        scalar1=kq_bias[:, 1:2], scalar2=None, op0=ALU.add,
                    )

            def emit_v(cb, use_act=False):
                # V as [hid, keys] (same shape as K), bias folded into the
                # psum->SBUF copy, then per-128 XBAR transposes into v_bf
                vp = psD.tile([128, 512], f32, tag="simd", name=f"vp{cb}")[:, :]
                for t in range(CT):
                    nc.tensor.matmul(
                        vp, wq_sc[:, t, 2 * HID : 3 * HID],
                        x_sb[:, t, cb * 512 : (cb + 1) * 512],
                        start=(t == 0), stop=(t == CT - 1),
                    )
                c_sl = slice(cb * 512, (cb + 1) * 512)
                if use_act:
                    nc.scalar.activation(
                        vh_bf[:, c_sl], vp, AF.Identity, bias=kq_bias[:, 2:3]
                    )
                else:
                    nc.vector.tensor_scalar(
                        vh_bf[:, c_sl], vp,
                        scalar1=kq_bias[:, 2:3], scalar2=None, op0=ALU.add,
                    )
                for kb in range(4 * cb, 4 * cb + 4):
                    nc.sync.dma_start_transpose(
                        v_bf[:, kb, :], vh_bf[:, kb * 128 : (kb + 1) * 128]
                    )
            zero_sb = big.tile([1, 512], bf16)
            nc.vector.memset(zero_sb[:], 0.0)
            emit_k(0)
            emit_q(0)


            # ---------------- attention + out-projection ----------------
            # Segment = (query block of 512, head pair): 64 slices (kb = i//2,
            # head = 2*pair + i%2), each slice one [128 keys x 512 q] sim
            # plane. Sim tiles hold 3 (or 2) planes in the 6-bank psA pool and
            # are consumed whole either by ScalarE (exact exp) or VectorE
            # (one-op Schraudolph exp in bf16 bit space, int16 out) -- bigger
            # tiles amortize each engine's fixed access cost. attn@V: per
            # slice 4 matmuls (one per 128-query block), exp weights
            # stationary, V^T_ext (32 V + 1 ones col) moving:
            # psO[q, 66*qs+33*hi] += e[:, qs]^T @ v_ext accumulated over kb;
            # one pair's worth fits a single PSUM bank.
            SLICES = NKB * 2  # 64 per segment

            def seg_pattern(npair):
                # npair 2-slice ScalarE exp tiles (psS) interleaved with
                # 1-slice VectorE Schraudolph tiles (psD); each stream is
                # independently double-buffered. Head tiles feed ScalarE (the
                # epilogue owns VectorE's stream head); tail tiles feed
                # VectorE so the next epilogue is not gated on ScalarE's lag.
                nd = SLICES - 2 * npair
                pat, a, d = ["A", "A"], 2, 0
                while a < npair or d < nd - 2:
                    if (d * npair >= a * nd and a < npair) or d >= nd - 2:
                        pat.append("A")
                        a += 1
                    else:
                        pat.append("D")
                        d += 1
                pat += ["D", "D"]
                return pat

            def make_epilogue(qb, pair, oacc, hT):
                # per qsub: 1/denominator (on the query's own partition),
                # broadcast-multiply O^T into this pair's half of hT
                def _emit(zmm):
                    ov = oacc[:, 0 : 4 * 66].rearrange("p (q c) -> p q c", c=66)
                    rcp = work.tile([128, 4, 2], f32, tag="rcp")
                    with nc.allow_low_precision(
                        reason="softmax denom reciprocal feeds bf16 output"
                    ):
                        nc.vector.reciprocal(rcp[:], ov[:, :, 64:66])
                        nc.vector.tensor_tensor(
                            hT[:, :, 64 * pair : 64 * pair + 64].rearrange(
                                "p q (h c) -> p q h c", c=DH
                            ),
                            ov[:, :, 0:64].rearrange(
                                "p q (h c) -> p q h c", c=DH
                            ),
                            rcp[:].unsqueeze(3).broadcast_to([128, 4, 2, DH]),
                            ALU.mult,
                        )
                    if zmm is not None:
                        zmm()
                return _emit

            def make_qb_transp(qb, hT):
                def _emit(zmm):
                    h_sb = hpool.tile([128, 512], bf16, tag="h")
                    for qs in range(4):
                        nc.sync.dma_start_transpose(
                            h_sb[:, qs * 128 : (qs + 1) * 128], hT[:, qs]
                        )
                    return h_sb
                return _emit

            def make_qb_mms(qb, h_box, yp_box):
                # out-projection over the transposed h (emitted a few tiles
                # after the transposes so their DMA latency stays off the PE
                # FIFO's critical path)
                def _emit(zmm):
                    h_sb = h_box[0]
                    if CFG["yp_psS"]:
                        ypt = psS.tile([128, 2, 512], f32, tag="sim", name=f"yp{qb}")
                        yps = [ypt[:, oc, :] for oc in range(CT)]
                    else:
                        yps = [
                            psD.tile([128, 512], f32, tag="simd", name=f"yp{qb}_{oc}")[:, :]
                            for oc in range(CT)
                        ]
                    yp_box.append(yps)
                    for qs in range(4):
                        for oc in range(CT):
                            nc.tensor.matmul(
                                yps[oc][:, qs * 128 : (qs + 1) * 128],
                                wo_bf[:, oc * 128 : (oc + 1) * 128],
                                h_sb[:, qs * 128 : (qs + 1) * 128],
                                start=True, stop=True,
                            )
                return _emit

            def make_qb_tail(qb, yp_box, use_act):
                def _emit(zmm):  # zmm unused: only the epilogue re-zeroes psO
                    yps = yp_box[0]
                    q0 = qb * 512
                    if use_act and CFG["tail_split"]:
                        # tail: ScalarE is idle after the last exp; halves let
                        # the output DMA start while qs2/3 are still in flight
                        yts = [
                            work.tile([128, 512], bf16, tag=f"yt{oc}", name=f"yt{oc}")
                            for oc in range(CT)
                        ]
                        for half in range(2):
                            c_sl = slice(half * 256, half * 256 + 256)
                            for oc in range(CT):
                                nc.scalar.activation(
                                    yts[oc][:, c_sl], yps[oc][:, c_sl],
                                    AF.Identity, bias=gb_sb[:, oc, 2:3],
                                )
                                nc.sync.dma_start(
                                    out_d[oc, :, q0 + half * 256 : q0 + half * 256 + 256],
                                    yts[oc][:, c_sl],
                                )
                    else:
                        for oc in range(CT):
                            yt = work.tile([128, 512], bf16, tag="yt")
                            if use_act:
                                nc.scalar.activation(
                                    yt[:], yps[oc][:], AF.Identity,
                                    bias=gb_sb[:, oc, 2:3],
                                )
                            else:
                                nc.vector.tensor_scalar(
                                    yt[:], yps[oc][:], scalar1=gb_sb[:, oc, 2:3],
                                    scalar2=None, op0=ALU.add,
                                )
                            nc.sync.dma_start(
                                out_d[oc, :, q0 : q0 + 512], yt[:]
                            )
                return _emit

            pending = []    # deferred epilogue / qb-tail emitters
            oaccq = []      # attn@V emitters lagging the exp stream

            def emit_sim(s_idx, tile_ap, plane, qb, pair):
                kb, hi = s_idx // 2, s_idx % 2
                h = 2 * pair + hi
                q_sl = slice(qb * 512, (qb + 1) * 512)
                nc.tensor.matmul(
                    tile_ap[:, plane, :],
                    k_bf[32 * h : 32 * h + 32, kb * 128 : (kb + 1) * 128],
                    q_bf[32 * h : 32 * h + 32, q_sl],
                    start=True, stop=True,
                    tile_position=(32 * h, 0),
                )

            def make_oacc(s_idx, e_ap, oacc, pair):
                kb, hi = s_idx // 2, s_idx % 2
                h = 2 * pair + hi
                def _emit(last=False):
                    # PSUM adds commute, so attn@V order across slices is
                    # free; stop (clearing the bank's group-started state)
                    # must ride the final matmul actually emitted
                    for qs in range(4):
                        col = 66 * qs + 32 * hi
                        nc.tensor.matmul(
                            oacc[:, col : col + 32],
                            e_ap[:, qs * 128 : (qs + 1) * 128],
                            v_bf[:, kb, 32 * h : 32 * h + 32],
                            start=False, stop=False,
                        )
                        nc.tensor.matmul(
                            oacc[:, 66 * qs + 64 + hi : 66 * qs + 65 + hi],
                            e_ap[:, qs * 128 : (qs + 1) * 128],
                            ones1[:],
                            start=False,
                            stop=(last and qs == 3),
                        )
                return _emit

            hT = None
            for sg in range(2 * NQB):
                qb, pair = sg // 2, sg % 2
                seg0 = sg == 0
                oacc = psO.tile([128, 512], f32, tag="oacc", name=f"o{sg}")

                def zmm(oacc=oacc):
                    # one start=True matmul filling the whole bank resets its
                    # pending-zero state in one shot, so the interleaved
                    # per-(head, qsub) groups can then pure-accumulate (a
                    # start per group would wipe the others' first block)
                    nc.tensor.matmul(
                        oacc[:], zero_sb[:, 0:128], zero_sb[:],
                        start=True, stop=False,
                    )

                if seg0:
                    zmm()
                if pair == 0:
                    hT = hpool.tile([128, 4, HID], bf16, tag="hT")
                pat = seg_pattern(CFG["np0"] if seg0 else CFG["np"])
                i = 0
                for ci, kind in enumerate(pat):
                    ts_ = 2 if kind == "A" else 1
                    if kind == "A":
                        tile = psS.tile([128, 2, 512], f32, tag="sim", name="sim")
                    else:
                        tile = psD.tile([128, 512], f32, tag="simd", name="simd").unsqueeze(1)
                    for s in range(ts_):
                        emit_sim(i + s, tile, s, qb, pair)
                    due = [
                        (o, fn) for o, fn in pending
                        if ci >= CFG["flush_ci"] + o
                    ]
                    if due:
                        # previous segment's epilogue: its DVE ops lead the
                        # queue; the freed psO bank is re-zeroed (zmm) right
                        # after its last read retires
                        for o, fn in due:
                            fn(zmm if not seg0 else None)
                        pending = [p for p in pending if p not in due]
                    if seg0:
                        # produce the next V / K column blocks, overlapped
                        # with the exp stream (V leads: its transposes add
                        # DMA latency before the attn@V needs it)
                        for kb in range((i + 1) // 2, (i + ts_ + 1) // 2):
                            if kb % 4 == 0 and 0 <= kb // 4 < 8:
                                emit_v(kb // 4, use_act=(kb // 4 < CFG["vact"]))
                            if kb % 4 == 2 and 0 < kb // 4 + 1 < 8:
                                emit_k(kb // 4 + 1)
                    if qb < NQB - 1 and pair == 1 and ci == 10:
                        emit_q(qb + 1)
                    if kind == "A":
                        e = epool.tile([128, 2, 512], bf16, tag="e")
                        nc.scalar.activation(e[:], tile[:], AF.Exp)
                        for s in range(2):
                            oaccq.append(make_oacc(i + s, e[:, s, :], oacc, pair))
                    else:
                        e2 = e2pool.tile([128, 512], mybir.dt.int16, tag="e2")
                        with nc.allow_low_precision(
                            reason="Schraudolph bf16 exp on DVE; softmax renormalizes"
                        ):
                            nc.vector.tensor_scalar(
                                e2[:], tile[:, 0, :],
                                scalar1=128.0 / 0.6931471805599453,
                                scalar2=(127.0 - 0.043) * 128.0,
                                op0=ALU.mult, op1=ALU.add,
                            )
                        oaccq.append(make_oacc(i, e2.bitcast(bf16), oacc, pair))
                    i += ts_
                    while len(oaccq) > CFG["oacc_hi"]:
                        oaccq.pop(0)()
                while oaccq:
                    fn_, last_ = oaccq.pop(0), not oaccq
                    fn_(last_)
                if sg == 2 * NQB - 1:
                    # tail: per-qsub normalize + immediate XBAR transpose so
                    # the HWDGE-serial transposes overlap the remaining norms
                    def final_epi(zmm, oacc=oacc, hT=hT, qb=qb):
                        h_sb = hpool.tile([128, 512], bf16, tag="h")
                        for qs in range(4):
                            rcp = work.tile([128, 2], f32, tag="rcp")
                            with nc.allow_low_precision(
                                reason="softmax denom reciprocal, bf16 output"
                            ):
                                nc.vector.reciprocal(
                                    rcp[:], oacc[:, 66 * qs + 64 : 66 * qs + 66]
                                )
                                nc.vector.tensor_tensor(
                                    hT[:, qs, 64 : 128].rearrange(
                                        "p (h c) -> p h c", c=DH
                                    ),
                                    oacc[:, 66 * qs : 66 * qs + 64].rearrange(
                                        "p (h c) -> p h c", c=DH
                                    ),
                                    rcp[:].unsqueeze(2).broadcast_to([128, 2, DH]),
                                    ALU.mult,
                                )
                            nc.sync.dma_start_transpose(
                                h_sb[:, qs * 128 : (qs + 1) * 128], hT[:, qs]
                            )
                        return h_sb
                    pending.append((0, lambda z, h_box2=None: None))
                    pending.pop()
                    h_box = []
                    pending.append(
                        (0, lambda z, fe=final_epi, h_box=h_box: h_box.append(fe(z)))
                    )
                    pending.append((0, make_qb_mms(qb, h_box, yp_box2 := [])))
                    pending.append((0, make_qb_tail(qb, yp_box2, use_act=True)))
                    continue
                pending.append((0, make_epilogue(qb, pair, oacc, hT)))
                if pair == 1:
                    h_box, yp_box = [], []
                    tp = make_qb_transp(qb, hT)
                    pending.append(
                        (0, lambda z, tp=tp, h_box=h_box: h_box.append(tp(None)))
                    )
                    pending.append((CFG["tail_lag"], make_qb_mms(qb, h_box, yp_box)))
                    pending.append(
                        (CFG["yt_lag"], make_qb_tail(qb, yp_box, use_act=(qb == NQB - 1)))
                    )
            for _, fn in pending:
                fn(None)
    return nc


def _prep_shared(w_qkv, w_out, b_out, gamma, beta):
    scale = DH ** -0.5
    wqkvT = np.ascontiguousarray(w_qkv.T).astype(np.float32).copy()  # [C, 384]
    wqkvT[:, :HID] *= scale
    wq = np.ascontiguousarray(wqkvT.reshape(CT, 128, 3 * HID))
    wo = np.ascontiguousarray(w_out.T).astype(np.float32)            # [HID, C]
    gb = np.stack(
        [
            np.asarray(gamma, np.float32).reshape(CT, 128).T,
            np.asarray(beta, np.float32).reshape(CT, 128).T,
            np.asarray(b_out, np.float32).reshape(CT, 128).T,
        ],
        axis=-1,
    )  # [128, CT, 3]
    gmask = np.zeros((128, CT, G), np.float32)
    sel = np.zeros((G, CT, 128), np.float32)
    for t in range(CT):
        for p in range(128):
            g = (t * 128 + p) // (C // G)
            gmask[p, t, g] = 1.0 / ((C // G) * N)
            sel[g, t, p] = 1.0
    return wq, wo, gb, gmask, sel


def _run(inputs, trace=False):
    from concourse.bass_utils import run_bass_kernel_spmd

    x = np.asarray(inputs["x"], np.float32)
    wq, wo, gb, gmask, sel = _prep_shared(
        np.asarray(inputs["w_qkv"], np.float32),
        np.asarray(inputs["w_out"], np.float32),
        np.asarray(inputs["b_out"], np.float32),
        np.asarray(inputs["gamma"], np.float32),
        np.asarray(inputs["beta"], np.float32),
    )
    if "nc" not in _BUILT:
        _BUILT["nc"] = build_nc()
    nc = _BUILT["nc"]

    in_maps = []
    for core in range(8):
        b_idx, qh = core // 2, core % 2
        xb = x[b_idx].reshape(C, N)
        if qh:
            xb = np.roll(xb, -NQ, axis=1)
        in_maps.append(
            {
                "x": np.ascontiguousarray(
                    xb.reshape(CT, 128, N).astype(ml_dtypes.bfloat16)
                ),
                "wq": wq, "wo": wo, "gb": gb, "gmask": gmask, "selT": sel,
            }
        )
    res = run_bass_kernel_spmd(
        nc, in_maps, core_ids=list(range(8)), trace=trace
    )
    out = np.empty((B, C, N), np.float32)
    for core in range(8):
        b_idx, qh = core // 2, core % 2
        y = res.results[core]["out"].astype(np.float32).reshape(C, NQ)
        out[b_idx, :, qh * NQ : (qh + 1) * NQ] = y
    return out.reshape(B, C, HW, HW), res


def kernel(**inputs) -> np.ndarray:
    out, _ = _run(inputs, trace=False)
    return out
